# revision 1
# baseline (speedup 1.0000x reference)
"""Trainium2 Bass kernel for nn_BasicBlock (spiking CNN block).

Sharding: data-parallel over batch B across 8 NeuronCores (4 batch x 4
timesteps = 16 images per core); BN batch stats via tiny AllReduce.

Per core:
- conv1: 3x3 taps as TensorEngine matmuls in fp16 hi/lo split arithmetic
  (~fp32 accuracy at bf16 speed): per tap [W1hi;W1hi] x [xhi;xlo] (K=128)
  + W1lo x xhi (K=64). Two images run concurrently via PE column tiling.
- BN stats (sum / sum-of-squares) accumulated during PSUM evacuation
  (ScalarE copy w/ accum_out; DVE square pass), all-reduced across cores.
- PLIF scan in "q-space" (conv-output units): BN scale/bias folded into
  per-channel threshold theta / constants, so no per-element BN apply.
- conv2 consumes exact 0/1 spikes in fp16: per tap [W2hi;W2lo] x [s1;s1]
  (K=128) gives both split terms in one matmul.
- Residual + LIF2 streamed in half-strips; out written via casting DMA.
"""
import sys
sys.path.insert(0, '/opt/trn_rl_repo')

import numpy as np

T, B, C, H, W = 4, 32, 64, 56, 56
NCORES = 8
BL = B // NCORES            # 4 local batch samples
NIMG = T * BL               # 16 images per core
HP = W + 2                  # 58
PP = HP * HP                # 3364 padded pixels
PIX = H * W                 # 3136
NCH = 7                     # conv chunks per image (8 rows each)
CHW = 8 * W                 # 448
NPAIR = 8                   # image pairs per core
EPS = 1e-5
NG = float((T * B) * PIX)   # 401408
QL = 14 * W                 # LIF quarter-strip length (784)
NQ = 4

_prog_cache = {}
DBG = False
NO_CC = False
PHASES = 3
TRACE = False
LAST_RES = None
LAST_NAMES = None
LAST_EXEC_NS = None


def _build(alpha1, alpha2):
    import concourse.mybir as mybir
    import concourse.tile as tile
    from concourse import bacc

    F32 = mybir.dt.float32
    F16 = mybir.dt.float16
    AO = mybir.AluOpType
    AF = mybir.ActivationFunctionType
    AX = mybir.AxisListType

    nc = bacc.Bacc(None, target_bir_lowering=False)
    names = {}

    with tile.TileContext(nc) as tc:
        with tc.tile_pool(name="dram", bufs=1, space="DRAM") as dram:
            xta = dram.tile([NIMG, 2, 64, PP], F16, kind="ExternalInput")
            xin = dram.tile([NIMG, 64, PIX], F32, kind="ExternalInput")
            w1a = dram.tile([128, 9 * 64], F16, kind="ExternalInput")
            w1b = dram.tile([128, 9 * 64], F16, kind="ExternalInput")
            w2a = dram.tile([128, 9 * 64], F16, kind="ExternalInput")
            cpar = dram.tile([128, 8], F32, kind="ExternalInput")
            outp = dram.tile([NIMG, 64, PIX], F32, kind="ExternalOutput")
            names.update(xta=xta.name, xin=xin.name, w1a=w1a.name,
                         w1b=w1b.name, w2a=w2a.name, cpar=cpar.name,
                         outp=outp.name)
            if DBG:
                y1d = dram.tile([NPAIR, 128, PIX], F32, kind="ExternalOutput")
                y2d = dram.tile([NPAIR, 128, PIX], F32, kind="ExternalOutput")
                s1d = dram.tile([NPAIR, 128, PIX], F32, kind="ExternalOutput")
                vecd = dram.tile([128, 8], F32, kind="ExternalOutput")
                names.update(y1d=y1d.name, y2d=y2d.name, s1d=s1d.name,
                             vecd=vecd.name)

            with tc.tile_pool(name="dramw", bufs=1, space="DRAM") as dramw, \
                 tc.tile_pool(name="wsb", bufs=1) as wsb, \
                 tc.tile_pool(name="ys", bufs=8) as yspool, \
                 tc.tile_pool(name="plane", bufs=4) as plpool, \
                 tc.tile_pool(name="hfp", bufs=2) as hf, \
                 tc.tile_pool(name="tiny", bufs=40) as tiny, \
                 tc.tile_pool(name="ps", bufs=7, space="PSUM") as ps:

                # ---- static parameter loads
                w1as = wsb.tile([128, 9 * 64], F16, tag="w1a")
                nc.sync.dma_start(w1as[:], w1a[:])
                w1bs = wsb.tile([128, 9 * 64], F16, tag="w1b")
                nc.sync.dma_start(w1bs[:], w1b[:])
                w2as = wsb.tile([128, 9 * 64], F16, tag="w2a")
                nc.sync.dma_start(w2as[:], w2a[:])
                cpars = wsb.tile([128, 8], F32, tag="cpar")
                nc.sync.dma_start(cpars[:], cpar[:])
                sums1 = wsb.tile([128, 56], F32, tag="sums1")
                sums1q = wsb.tile([128, 56], F32, tag="sums1q")
                sums2 = wsb.tile([128, 56], F32, tag="sums2")
                sums2q = wsb.tile([128, 56], F32, tag="sums2q")
                if PHASES < 2:
                    nc.vector.memset(sums2[:], 0.0)
                    nc.vector.memset(sums2q[:], 0.0)

                def conv_img_pair(plA, plB, lhi, llo, dst_strip, sums_t,
                                  sumsq_t, pcol):
                    """One image pair -> 7 chunks in two waves (4+3); taps
                    outer within a wave so consecutive matmuls hit different
                    PSUM banks and weight loads amortize; wave evacuations
                    overlap the next wave's matmuls."""
                    plAr = plA.rearrange("p (r w) -> p r w", w=HP)
                    plBr = plB.rearrange("p (r w) -> p r w", w=HP)
                    for wave in (range(0, 4), range(4, 7)):
                        pts = {}
                        for cth in wave:
                            pts[cth] = ps.tile([128, CHW], F32, tag="ps",
                                               bufs=7, name=f"psum{cth}")
                        last_a = 8 if llo is None else None
                        for a in range(9):
                            di, dj = a // 3, a % 3
                            for cth in wave:
                                r0 = 8 * cth + di
                                for j, plr in enumerate((plAr, plBr)):
                                    rhs = plr[:, r0:r0 + 8, dj:dj + W]
                                    out = pts[cth][64 * j:64 * (j + 1), :] \
                                        .rearrange("p (r w) -> p r w", r=8)
                                    nc.tensor.matmul(
                                        out, lhi[:, a * 64:(a + 1) * 64], rhs,
                                        start=(a == 0), stop=(a == last_a),
                                        tile_position=(0, 64 * j),
                                        skip_group_check=True)
                        if llo is not None:
                            for a in range(9):
                                di, dj = a // 3, a % 3
                                for cth in wave:
                                    r0 = 8 * cth + di
                                    for j, plr in enumerate((plAr, plBr)):
                                        rhs = plr[:, r0:r0 + 8, dj:dj + W]
                                        out = pts[cth][64 * j:64 * (j + 1), :] \
                                            .rearrange("p (r w) -> p r w", r=8)
                                        nc.tensor.matmul(
                                            out, llo[:, a * 64:(a + 1) * 64], rhs,
                                            start=False, stop=(a == 8),
                                            tile_position=(0, 64 * j),
                                            skip_group_check=True)
                        for cth in wave:
                            nc.scalar.activation(
                                dst_strip[:, CHW * cth:CHW * (cth + 1)],
                                pts[cth][:], AF.Copy,
                                accum_out=sums_t[:, pcol * 7 + cth:pcol * 7 + cth + 1])
                            jk = ps.tile([128, CHW], F32, tag="psjk", bufs=1,
                                         name="psjk")
                            sl = dst_strip[:, CHW * cth:CHW * (cth + 1)]
                            nc.vector.scalar_tensor_tensor(
                                jk[:], sl, 1.0, sl, AO.bypass, AO.mult,
                                accum_out=sumsq_t[:, pcol * 7 + cth:pcol * 7 + cth + 1])

                # ================= phase A: conv1 =================
                y1s = []
                for p in range(NPAIR):
                    tt_, bp = p // 2, p % 2
                    iA = tt_ * 4 + bp * 2
                    planes = []
                    for j in range(2):
                        i = iA + j
                        ta = plpool.tile([128, PP], F16, tag="ta")
                        nc.sync.dma_start(ta[0:64, :], xta[i, 0])
                        nc.sync.dma_start(ta[64:128, :], xta[i, 1])
                        planes.append(ta)
                    strip = yspool.tile([128, PIX], F32, tag="ys")
                    y1s.append(strip)
                    conv_img_pair(planes[0], planes[1], w1as, w1bs, strip,
                                  sums1, sums1q, p)
                    if DBG:
                        nc.sync.dma_start(y1d[p], strip[:])

                # ---- stats1 allreduce
                cc1i = dramw.tile([128, 2], F32)
                cc1o = dramw.tile([128, 2], F32, addr_space="Shared")
                acc1 = tiny.tile([128, 2], F32, tag="acc")
                nc.vector.tensor_reduce(acc1[:, 0:1], sums1[:], AX.X, AO.add)
                nc.vector.tensor_reduce(acc1[:, 1:2], sums1q[:], AX.X, AO.add)
                nc.sync.dma_start(cc1i[:], acc1[:])
                if NO_CC:
                    nc.sync.dma_start(cc1o[:], cc1i[:])
                else:
                    nc.gpsimd.collective_compute(
                        "AllReduce", AO.add, ins=[cc1i[:]], outs=[cc1o[:]],
                        replica_groups=[list(range(NCORES))])
                g1 = tiny.tile([128, 2], F32, tag="acc")
                nc.sync.dma_start(g1[:], cc1o[:])

                shuf_mask = [(i + 16) % 32 for i in range(32)]

                def stats_block(g, gamma, beta, rga, rgam, alpha):
                    gr = tiny.tile([128, 2], F32, tag="acc")
                    nc.sync.dma_start(gr[0:64, :], g[64:128, :])
                    nc.sync.dma_start(gr[64:128, :], g[0:64, :])
                    tot = tiny.tile([128, 2], F32, tag="acc")
                    nc.vector.tensor_tensor(tot[:], g[:], gr[:], AO.add)
                    mean = tiny.tile([128, 1], F32, tag="t1")
                    nc.vector.tensor_scalar(mean[:], tot[:, 0:1], 1.0 / NG,
                                            None, AO.mult)
                    msq = tiny.tile([128, 1], F32, tag="t1")
                    nc.vector.tensor_scalar(msq[:], tot[:, 1:2], 1.0 / NG,
                                            None, AO.mult)
                    m2 = tiny.tile([128, 1], F32, tag="t1")
                    nc.vector.scalar_tensor_tensor(m2[:], mean[:], 1.0, mean[:],
                                                   AO.bypass, AO.mult)
                    var = tiny.tile([128, 1], F32, tag="t1")
                    nc.vector.tensor_tensor(var[:], msq[:], m2[:], AO.subtract)
                    epst = tiny.tile([128, 1], F32, tag="t1")
                    nc.vector.memset(epst[:], EPS)
                    std = tiny.tile([128, 1], F32, tag="t1")
                    nc.scalar.activation(std[:], var[:], AF.Sqrt, bias=epst[:])
                    rstd = tiny.tile([128, 1], F32, tag="t1")
                    nc.vector.reciprocal(rstd[:], std[:])
                    sc = tiny.tile([128, 1], F32, tag="t1")
                    nc.vector.tensor_tensor(sc[:], gamma, rstd[:], AO.mult)
                    nmsc = tiny.tile([128, 1], F32, tag="t1")
                    nc.vector.scalar_tensor_tensor(nmsc[:], mean[:], -1.0, sc[:],
                                                   AO.mult, AO.mult)
                    bi = tiny.tile([128, 1], F32, tag="t1")
                    nc.vector.tensor_tensor(bi[:], beta, nmsc[:], AO.add)
                    stdrg = tiny.tile([128, 1], F32, tag="t1")
                    nc.vector.tensor_tensor(stdrg[:], std[:], rga, AO.mult)
                    nbst = tiny.tile([128, 1], F32, tag="t1")
                    nc.vector.scalar_tensor_tensor(nbst[:], bi[:], -alpha,
                                                   stdrg[:], AO.mult, AO.mult)
                    th = tiny.tile([128, 1], F32, tag="t1")
                    nc.vector.tensor_tensor(th[:], stdrg[:], nbst[:], AO.add)
                    bstd = tiny.tile([128, 1], F32, tag="t1")
                    nc.vector.tensor_tensor(bstd[:], bi[:], std[:], AO.mult)
                    gamv = tiny.tile([128, 1], F32, tag="t1")
                    nc.vector.tensor_tensor(gamv[:], bstd[:], rgam, AO.mult)
                    rscv = tiny.tile([128, 1], F32, tag="t1")
                    nc.vector.tensor_tensor(rscv[:], std[:], rgam, AO.mult)
                    gmw = tiny.tile([128, 1], F32, tag="t1")
                    nc.vector.tensor_scalar(gmw[:], gamv[:], 1.0 - alpha, None,
                                            AO.mult)
                    return th, gamv, rscv, gmw

                th1, gm1, _rsc1, gmw1 = stats_block(
                    g1, cpars[:, 0:1], cpars[:, 1:2], cpars[:, 4:5],
                    cpars[:, 6:7], alpha1)
                if DBG:
                    nc.sync.dma_start(vecd[:, 0:1], th1[:])
                    nc.sync.dma_start(vecd[:, 1:2], gm1[:])
                    nc.sync.dma_start(vecd[:, 4:5], acc1[:, 0:1])
                    nc.sync.dma_start(vecd[:, 5:6], acc1[:, 1:2])

                # ============ phase B + C: LIF1 + conv2 ============
                y2s = [None] * NPAIR
                for bp in range(2 if PHASES >= 2 else 0):
                    Pprev = [None] * NQ
                    for t in range(1, 5):
                        p = (t - 1) * 2 + bp
                        s1tq = []
                        for hq in range(NQ):
                            off = QL * hq
                            ysl = y1s[p][:, off:off + QL]
                            if t == 1:
                                qa = ysl
                            else:
                                q = hf.tile([128, QL], F32, tag="q2", bufs=4)
                                nc.gpsimd.tensor_tensor(q[:], ysl,
                                                        Pprev[hq][:], AO.add)
                                qa = q[:]
                            s1t = hf.tile([128, QL], F16, tag="s1t", bufs=4)
                            nc.vector.tensor_scalar(s1t[:], qa, th1[:],
                                                    None, AO.is_ge)
                            s1tq.append(s1t)
                            if DBG:
                                nc.gpsimd.dma_start(
                                    s1d[p, :, off:off + QL], s1t[:])
                            if t < 4:
                                sb = hf.tile([128, QL], F16, tag="sb", bufs=2)
                                nc.vector.tensor_scalar(sb[:], qa, th1[:],
                                                        None, AO.is_lt)
                                wv = hf.tile([128, QL], F32, tag="q2", bufs=4)
                                nc.vector.tensor_scalar(
                                    wv[:], qa, gm1[:], 1.0 - alpha1,
                                    AO.add, AO.mult)
                                Pn = hf.tile([128, QL], F32, tag="pp", bufs=6)
                                nc.vector.tensor_tensor(Pn[:], wv[:], sb[:],
                                                        AO.mult)
                                Pprev[hq] = Pn
                        iA = (t - 1) * 4 + bp * 2
                        tas_pair = []
                        for j in range(2):
                            tas = plpool.tile([128, PP], F16, tag="ta")
                            tasr = tas.rearrange("p (r w) -> p r w", w=HP)
                            nc.gpsimd.memset(tas[:, 0:HP], 0.0)
                            nc.gpsimd.memset(tas[:, PP - HP:PP], 0.0)
                            nc.gpsimd.memset(tasr[:, :, 0:1], 0.0)
                            nc.gpsimd.memset(tasr[:, :, HP - 1:HP], 0.0)
                            for hq in range(NQ):
                                src = s1tq[hq][64 * j:64 * (j + 1), :] \
                                    .rearrange("p (r w) -> p r w", w=W)
                                dsti = tasr[:, 1 + 14 * hq:1 + 14 * (hq + 1),
                                            1:1 + W]
                                nc.sync.dma_start(dsti[0:64], src)
                                nc.sync.dma_start(dsti[64:128], src)
                            tas_pair.append(tas)
                        strip2 = yspool.tile([128, PIX], F32, tag="ys")
                        y2s[p] = strip2
                        conv_img_pair(tas_pair[0], tas_pair[1], w2as, None,
                                      strip2, sums2, sums2q, p)
                        if DBG:
                            nc.sync.dma_start(y2d[p], strip2[:])

                # ---- stats2 allreduce
                cc2i = dramw.tile([128, 2], F32)
                cc2o = dramw.tile([128, 2], F32, addr_space="Shared")
                acc2 = tiny.tile([128, 2], F32, tag="acc")
                nc.vector.tensor_reduce(acc2[:, 0:1], sums2[:], AX.X, AO.add)
                nc.vector.tensor_reduce(acc2[:, 1:2], sums2q[:], AX.X, AO.add)
                nc.sync.dma_start(cc2i[:], acc2[:])
                if NO_CC:
                    nc.sync.dma_start(cc2o[:], cc2i[:])
                else:
                    nc.gpsimd.collective_compute(
                        "AllReduce", AO.add, ins=[cc2i[:]], outs=[cc2o[:]],
                        replica_groups=[list(range(NCORES))])
                g2 = tiny.tile([128, 2], F32, tag="acc")
                nc.sync.dma_start(g2[:], cc2o[:])
                th2, gm2, rsc2, gmw2 = stats_block(
                    g2, cpars[:, 2:3], cpars[:, 3:4], cpars[:, 5:6],
                    cpars[:, 7:8], alpha2)
                if DBG:
                    nc.sync.dma_start(vecd[:, 2:3], th2[:])
                    nc.sync.dma_start(vecd[:, 3:4], gm2[:])

                # ============ phase D: residual + LIF2 ============
                for bp in range(2 if PHASES >= 3 else 0):
                    Pprev2 = [None] * NQ
                    for t in range(1, 5):
                        p = (t - 1) * 2 + bp
                        iA = (t - 1) * 4 + bp * 2
                        for hq in range(NQ):
                            off = QL * hq
                            xs = hf.tile([128, QL], F32, tag="xs", bufs=6)
                            nc.scalar.dma_start(xs[0:64, :],
                                                xin[iA, :, off:off + QL])
                            nc.scalar.dma_start(xs[64:128, :],
                                                xin[iA + 1, :, off:off + QL])
                            xsc = hf.tile([128, QL], F32, tag="xs", bufs=6)
                            nc.scalar.activation(xsc[:], xs[:], AF.Copy,
                                                 scale=rsc2[:])
                            r = hf.tile([128, QL], F32, tag="xs", bufs=6)
                            nc.gpsimd.tensor_tensor(
                                r[:], xsc[:], y2s[p][:, off:off + QL], AO.add)
                            if t == 1:
                                q2v = r[:]
                            else:
                                q2 = hf.tile([128, QL], F32, tag="q2", bufs=4)
                                nc.vector.tensor_tensor(q2[:], r[:],
                                                        Pprev2[hq][:], AO.add)
                                q2v = q2[:]
                            ot = hf.tile([128, QL], F32, tag="ot", bufs=2)
                            nc.vector.tensor_scalar(ot[:], q2v, th2[:],
                                                    None, AO.is_ge)
                            nc.sync.dma_start(outp[iA, :, off:off + QL],
                                              ot[0:64, :])
                            nc.sync.dma_start(outp[iA + 1, :, off:off + QL],
                                              ot[64:128, :])
                            if t < 4:
                                sb2 = hf.tile([128, QL], F16, tag="sb", bufs=2)
                                nc.vector.tensor_scalar(sb2[:], q2v, th2[:],
                                                        None, AO.is_lt)
                                wv2 = hf.tile([128, QL], F32, tag="q2", bufs=4)
                                nc.scalar.activation(wv2[:], q2v, AF.Identity,
                                                     bias=gmw2[:],
                                                     scale=1.0 - alpha2)
                                Pn = hf.tile([128, QL], F32, tag="pp", bufs=6)
                                nc.vector.tensor_tensor(Pn[:], wv2[:],
                                                        sb2[:], AO.mult)
                                Pprev2[hq] = Pn

    nc.compile()
    return nc, names


def _sigmoid(x):
    return 1.0 / (1.0 + np.exp(-float(x)))


def prepare(x, conv1_w, bn1_gamma, bn1_beta, lif1_w, conv2_w, bn2_gamma,
            bn2_beta, lif2_w):
    x = np.ascontiguousarray(np.asarray(x, np.float32))
    conv1_w = np.asarray(conv1_w, np.float32)
    conv2_w = np.asarray(conv2_w, np.float32)

    a1 = _sigmoid(np.asarray(lif1_w).reshape(-1)[0])
    a2 = _sigmoid(np.asarray(lif2_w).reshape(-1)[0])

    key = (round(a1, 12), round(a2, 12))
    if key not in _prog_cache:
        _prog_cache[key] = _build(a1, a2)
    nc, names = _prog_cache[key]

    # fp16 hi/lo split of x, padded planes (encoding only; exact split)
    xh = x.astype(np.float16)
    xl = (x - xh.astype(np.float32)).astype(np.float16)
    xpad = np.zeros((T, B, C, 2, HP, HP), np.float16)
    xpad[:, :, :, 0, 1:57, 1:57] = xh
    xpad[:, :, :, 1, 1:57, 1:57] = xl
    xpad = np.ascontiguousarray(xpad.transpose(0, 1, 3, 2, 4, 5))  # t,b,2,c,hp,hp

    w1h = conv1_w.astype(np.float16)
    w1l = (conv1_w - w1h.astype(np.float32)).astype(np.float16)
    w2h = conv2_w.astype(np.float16)
    w2l = (conv2_w - w2h.astype(np.float32)).astype(np.float16)

    def tapstack(wtop, wbot):
        out = np.zeros((128, 9 * 64), np.float16)
        for a in range(9):
            di, dj = a // 3, a % 3
            out[0:64, a * 64:(a + 1) * 64] = wtop[:, :, di, dj].T
            out[64:128, a * 64:(a + 1) * 64] = wbot[:, :, di, dj].T
        return out

    w1a_np = tapstack(w1h, w1h)
    w1b_np = tapstack(w1l, w1l)
    w2a_np = tapstack(w2h, w2l)

    def dup(v):
        v = np.asarray(v, np.float32).reshape(64)
        return np.concatenate([v, v])

    cpar_np = np.zeros((128, 8), np.float32)
    cpar_np[:, 0] = dup(bn1_gamma)
    cpar_np[:, 1] = dup(bn1_beta)
    cpar_np[:, 2] = dup(bn2_gamma)
    cpar_np[:, 3] = dup(bn2_beta)
    cpar_np[:, 4] = 1.0 / (a1 * dup(bn1_gamma))
    cpar_np[:, 5] = 1.0 / (a2 * dup(bn2_gamma))
    cpar_np[:, 6] = 1.0 / dup(bn1_gamma)
    cpar_np[:, 7] = 1.0 / dup(bn2_gamma)

    in_maps = []
    for k in range(NCORES):
        xta_np = np.ascontiguousarray(
            xpad[:, 4 * k:4 * k + 4].reshape(NIMG, 2, 64, PP))
        xin_np = np.ascontiguousarray(
            x[:, 4 * k:4 * k + 4].reshape(NIMG, 64, PIX))
        in_maps.append({
            names['xta']: xta_np,
            names['xin']: xin_np,
            names['w1a']: w1a_np,
            names['w1b']: w1b_np,
            names['w2a']: w2a_np,
            names['cpar']: cpar_np,
        })

    return nc, names, in_maps


def kernel(**inputs):
    from concourse.bass_utils import run_bass_kernel_spmd
    nc, names, in_maps = prepare(**inputs)
    res = run_bass_kernel_spmd(nc, in_maps, core_ids=list(range(NCORES)))
    global LAST_RES, LAST_NAMES
    LAST_RES, LAST_NAMES = res, names
    out = np.empty((T, B, C, H, W), np.float32)
    for k in range(NCORES):
        o = res.results[k][names['outp']]
        out[:, 4 * k:4 * k + 4] = o.reshape(T, BL, C, H, W)
    return out


if __name__ == "__main__":
    rng = np.random.default_rng(0)
    xs = rng.standard_normal((T, B, C, H, W)).astype(np.float32)
    w1 = (rng.standard_normal((64, 64, 3, 3)) * 0.05).astype(np.float32)
    w2 = (rng.standard_normal((64, 64, 3, 3)) * 0.05).astype(np.float32)
    o = kernel(xs, w1, np.ones(64, np.float32), np.zeros(64, np.float32),
               np.zeros(1, np.float32), w2, np.ones(64, np.float32),
               np.zeros(64, np.float32), np.zeros(1, np.float32))
    print("ran:", o.shape, float(o.mean()))



# revision 10
# speedup vs baseline: 1.1231x; 1.1231x over previous
"""Trainium2 Bass kernel for nn_BasicBlock (spiking CNN block).

Sharding: data-parallel over batch B across 8 NeuronCores (4 batch x 4
timesteps = 16 images per core); BN batch stats via tiny AllReduce.

Per core:
- conv1: 15 matmuls per chunk-image (vs naive 18): hi pass per tap
  [W1hi;W1hi] x [xhi;xlo] (K=128, 9 taps), lo pass tap-paired
  [W1lo(di,0);W1lo(di,1)] x [xhi; xhi<<1col] (3) plus solo W1lo(di,2) x
  xhi (K=64, 3).  Exact fp16 hi/lo split arithmetic (~fp32 accuracy).
- BN stats (sum / sum-of-squares) accumulated during PSUM evacuation
  (ScalarE copy + Square, both with accum_out), all-reduced across cores.
- PLIF scans in "q-space" (conv-output units): BN scale/bias folded into
  per-channel threshold/constants.
- LIF1 spikes written by the compare op directly into the padded conv2
  plane interiors; the [s;s] duplication is one full-plane DMA.
- conv2 consumes exact 0/1 spikes in fp16: per tap [W2hi;W2lo] x [s1;s1]
  (K=128) gives both split terms in one matmul.
- Residual + LIF2 in quarter strips with image-pair-merged DMAs; x
  prefetched on the SP queue during conv2; output written as fp16.
"""
import sys
sys.path.insert(0, '/opt/trn_rl_repo')

import numpy as np

T, B, C, H, W = 4, 32, 64, 56, 56
NCORES = 8
BL = B // NCORES            # 4 local batch samples
NIMG = T * BL               # 16 images per core
HP = W + 2                  # 58
PP = HP * HP                # 3364 padded pixels
PIX = H * W                 # 3136
NCH = 7                     # conv chunks per image (8 rows each)
CHW = 8 * W                 # 448
NPAIR = 8                   # image pairs per core
EPS = 1e-5
NG = float((T * B) * PIX)   # 401408
QL = 14 * W                 # LIF quarter-strip length (784)
NQ = 4

_prog_cache = {}
NO_CC = False
TRACE = False
LAST_RES = None
LAST_NAMES = None


def _build(alpha1, alpha2):
    import concourse.mybir as mybir
    import concourse.tile as tile
    from concourse import bacc

    F32 = mybir.dt.float32
    F16 = mybir.dt.float16
    AO = mybir.AluOpType
    AF = mybir.ActivationFunctionType
    AX = mybir.AxisListType

    nc = bacc.Bacc(None, target_bir_lowering=False)
    names = {}

    with tile.TileContext(nc) as tc:
        with tc.tile_pool(name="dram", bufs=1, space="DRAM") as dram:
            xta = dram.tile([NIMG, 2, 64, PP], F16, kind="ExternalInput")
            xtb = dram.tile([NIMG, 2, 64, PP], F16, kind="ExternalInput")
            xin = dram.tile([NIMG, 64, PIX], F32, kind="ExternalInput")
            w1a = dram.tile([128, 9 * 64], F16, kind="ExternalInput")
            w1p = dram.tile([128, 3 * 64], F16, kind="ExternalInput")
            w1s = dram.tile([128, 3 * 64], F16, kind="ExternalInput")
            w2a = dram.tile([128, 9 * 64], F16, kind="ExternalInput")
            cpar = dram.tile([128, 8], F32, kind="ExternalInput")
            outp = dram.tile([NIMG, 64, PIX], F16, kind="ExternalOutput")
            names.update(xta=xta.name, xtb=xtb.name, xin=xin.name,
                         w1a=w1a.name, w1p=w1p.name, w1s=w1s.name,
                         w2a=w2a.name, cpar=cpar.name, outp=outp.name)

            with tc.tile_pool(name="dramw", bufs=1, space="DRAM") as dramw, \
                 tc.tile_pool(name="wsb", bufs=1) as wsb, \
                 tc.tile_pool(name="ys", bufs=8) as yspool, \
                 tc.tile_pool(name="plane", bufs=5) as plpool, \
                 tc.tile_pool(name="plb", bufs=3) as plbpool, \
                 tc.tile_pool(name="hfp", bufs=2) as hf, \
                 tc.tile_pool(name="tiny", bufs=40) as tiny, \
                 tc.tile_pool(name="ps", bufs=7, space="PSUM") as ps:

                # ---- static parameter loads (Act queue; planes go on SP)
                w1as = wsb.tile([128, 9 * 64], F16, tag="w1a")
                nc.scalar.dma_start(w1as[:], w1a[:])
                w1ps = wsb.tile([128, 3 * 64], F16, tag="w1p")
                nc.scalar.dma_start(w1ps[:], w1p[:])
                w1ss = wsb.tile([128, 3 * 64], F16, tag="w1s")
                nc.scalar.dma_start(w1ss[:], w1s[:])
                w2as = wsb.tile([128, 9 * 64], F16, tag="w2a")
                nc.scalar.dma_start(w2as[:], w2a[:])
                cpars = wsb.tile([128, 8], F32, tag="cpar")
                nc.scalar.dma_start(cpars[:], cpar[:])
                sums1 = wsb.tile([128, 56], F32, tag="sums1")
                sums1q = wsb.tile([128, 56], F32, tag="sums1q")
                sums2 = wsb.tile([128, 56], F32, tag="sums2")
                sums2q = wsb.tile([128, 56], F32, tag="sums2q")

                def evac(pts, dst, sums_t, sumsq_t, col):
                    nc.scalar.activation(dst, pts[:], AF.Copy,
                                         accum_out=sums_t[:, col:col + 1])
                    nc.scalar.activation(pts[:], pts[:], AF.Square,
                                         accum_out=sumsq_t[:, col:col + 1])

                def conv1_pair(plA, plB, pbA, pbB, dst_strip, pcol):
                    """Conv1 for one image pair: per chunk 9 hi-pass K=128
                    matmuls ([W1hi;W1hi] x [xh;xl]) + 3 tap-paired lo
                    ([W1lo(di,0);W1lo(di,1)] x [xh;xh<<1]) + 3 solo K=64
                    (W1lo(di,2) x xh).  Waves of 4+3 chunks so evacuations
                    overlap the next wave."""
                    plAr = plA.rearrange("p (r w) -> p r w", w=HP)
                    plBr = plB.rearrange("p (r w) -> p r w", w=HP)
                    pbAr = pbA.rearrange("p (r w) -> p r w", w=HP)
                    pbBr = pbB.rearrange("p (r w) -> p r w", w=HP)
                    for wave in (range(0, 4), range(4, 7)):
                        pts = {}
                        for cth in wave:
                            pts[cth] = ps.tile([128, CHW], F32, tag="ps",
                                               bufs=7, name=f"psum{cth}")
                        for a in range(9):
                            di, dj = a // 3, a % 3
                            for cth in wave:
                                r0 = 8 * cth + di
                                for j, plr in enumerate((plAr, plBr)):
                                    out = pts[cth][64 * j:64 * (j + 1), :] \
                                        .rearrange("p (r w) -> p r w", r=8)
                                    nc.tensor.matmul(
                                        out, w1as[:, a * 64:(a + 1) * 64],
                                        plr[:, r0:r0 + 8, dj:dj + W],
                                        start=(a == 0), stop=False,
                                        tile_position=(0, 64 * j),
                                        skip_group_check=True)
                        for di in range(3):
                            for cth in wave:
                                r0 = 8 * cth + di
                                for j, pbr in enumerate((pbAr, pbBr)):
                                    out = pts[cth][64 * j:64 * (j + 1), :] \
                                        .rearrange("p (r w) -> p r w", r=8)
                                    nc.tensor.matmul(
                                        out, w1ps[:, di * 64:(di + 1) * 64],
                                        pbr[:, r0:r0 + 8, 0:W],
                                        start=False, stop=False,
                                        tile_position=(0, 64 * j),
                                        skip_group_check=True)
                        for di in range(3):
                            for cth in wave:
                                r0 = 8 * cth + di
                                for j, plr in enumerate((plAr, plBr)):
                                    out = pts[cth][64 * j:64 * (j + 1), :] \
                                        .rearrange("p (r w) -> p r w", r=8)
                                    nc.tensor.matmul(
                                        out, w1ss[0:64, di * 64:(di + 1) * 64],
                                        plr[0:64, r0:r0 + 8, 2:2 + W],
                                        start=False, stop=(di == 2),
                                        tile_position=(0, 64 * j),
                                        skip_group_check=True)
                        for cth in wave:
                            evac(pts[cth],
                                 dst_strip[:, CHW * cth:CHW * (cth + 1)],
                                 sums1, sums1q, pcol * 7 + cth)

                def conv2_pair(plA, plB, dst_strip, pcol):
                    plAr = plA.rearrange("p (r w) -> p r w", w=HP)
                    plBr = plB.rearrange("p (r w) -> p r w", w=HP)
                    for wave in (range(0, 4), range(4, 7)):
                        pts = {}
                        for cth in wave:
                            pts[cth] = ps.tile([128, CHW], F32, tag="ps",
                                               bufs=7, name=f"psum{cth}")
                        for a in range(9):
                            di, dj = a // 3, a % 3
                            for cth in wave:
                                r0 = 8 * cth + di
                                for j, plr in enumerate((plAr, plBr)):
                                    out = pts[cth][64 * j:64 * (j + 1), :] \
                                        .rearrange("p (r w) -> p r w", r=8)
                                    nc.tensor.matmul(
                                        out, w2as[:, a * 64:(a + 1) * 64],
                                        plr[:, r0:r0 + 8, dj:dj + W],
                                        start=(a == 0), stop=(a == 8),
                                        tile_position=(0, 64 * j),
                                        skip_group_check=True)
                        for cth in wave:
                            evac(pts[cth],
                                 dst_strip[:, CHW * cth:CHW * (cth + 1)],
                                 sums2, sums2q, pcol * 7 + cth)

                # ================= phase A: conv1 =================
                y1s = []
                for p in range(NPAIR):
                    tt_, bp = p // 2, p % 2
                    iA = tt_ * 4 + bp * 2
                    tas, pbs = [], []
                    for j in range(2):
                        i = iA + j
                        ta = plpool.tile([128, PP], F16, tag="pl")
                        nc.sync.dma_start(ta[:], xta[i])
                        tas.append(ta)
                        pb = plbpool.tile([128, PP], F16, tag="plb")
                        nc.sync.dma_start(pb[:], xtb[i])
                        pbs.append(pb)
                    strip = yspool.tile([128, PIX], F32, tag=f"ys{p}",
                                        bufs=1)
                    y1s.append(strip)
                    conv1_pair(tas[0], tas[1], pbs[0], pbs[1], strip, p)

                # ---- stats1 allreduce
                cc1i = dramw.tile([128, 2], F32)
                cc1o = dramw.tile([128, 2], F32, addr_space="Shared")
                acc1 = tiny.tile([128, 2], F32, tag="acc")
                nc.vector.tensor_reduce(acc1[:, 0:1], sums1[:], AX.X, AO.add)
                nc.vector.tensor_reduce(acc1[:, 1:2], sums1q[:], AX.X, AO.add)
                nc.sync.dma_start(cc1i[:], acc1[:])
                if NO_CC:
                    nc.sync.dma_start(cc1o[:], cc1i[:])
                else:
                    nc.gpsimd.collective_compute(
                        "AllReduce", AO.add, ins=[cc1i[:]], outs=[cc1o[:]],
                        replica_groups=[list(range(NCORES))])
                g1 = tiny.tile([128, 2], F32, tag="acc")
                nc.sync.dma_start(g1[:], cc1o[:])

                def stats_block(g, gamma, beta, rga, rgam, alpha):
                    gr = tiny.tile([128, 2], F32, tag="acc")
                    nc.sync.dma_start(gr[0:64, :], g[64:128, :])
                    nc.scalar.dma_start(gr[64:128, :], g[0:64, :])
                    tot = tiny.tile([128, 2], F32, tag="acc")
                    nc.vector.tensor_tensor(tot[:], g[:], gr[:], AO.add)
                    mm = tiny.tile([128, 2], F32, tag="acc")
                    nc.vector.tensor_scalar(mm[:], tot[:], 1.0 / NG, None,
                                            AO.mult)
                    mean = mm[:, 0:1]
                    m2 = tiny.tile([128, 1], F32, tag="t1")
                    nc.vector.tensor_tensor(m2[:], mean, mean, AO.mult)
                    var = tiny.tile([128, 1], F32, tag="t1")
                    nc.vector.tensor_tensor(var[:], mm[:, 1:2], m2[:],
                                            AO.subtract)
                    epst = tiny.tile([128, 1], F32, tag="t1")
                    nc.vector.memset(epst[:], EPS)
                    std = tiny.tile([128, 1], F32, tag="t1")
                    nc.scalar.activation(std[:], var[:], AF.Sqrt, bias=epst[:])
                    rstd = tiny.tile([128, 1], F32, tag="t1")
                    nc.vector.reciprocal(rstd[:], std[:])
                    sc = tiny.tile([128, 1], F32, tag="t1")
                    nc.vector.tensor_tensor(sc[:], gamma, rstd[:], AO.mult)
                    nmsc = tiny.tile([128, 1], F32, tag="t1")
                    nc.vector.scalar_tensor_tensor(nmsc[:], mean, -1.0, sc[:],
                                                   AO.mult, AO.mult)
                    bi = tiny.tile([128, 1], F32, tag="t1")
                    nc.vector.tensor_tensor(bi[:], beta, nmsc[:], AO.add)
                    stdrg = tiny.tile([128, 1], F32, tag="t1")
                    nc.vector.tensor_tensor(stdrg[:], std[:], rga, AO.mult)
                    nbst = tiny.tile([128, 1], F32, tag="t1")
                    nc.vector.scalar_tensor_tensor(nbst[:], bi[:], -alpha,
                                                   stdrg[:], AO.mult, AO.mult)
                    th = tiny.tile([128, 1], F32, tag="t1")
                    nc.vector.tensor_tensor(th[:], stdrg[:], nbst[:], AO.add)
                    bstd = tiny.tile([128, 1], F32, tag="t1")
                    nc.vector.tensor_tensor(bstd[:], bi[:], std[:], AO.mult)
                    gamv = tiny.tile([128, 1], F32, tag="t1")
                    nc.vector.tensor_tensor(gamv[:], bstd[:], rgam, AO.mult)
                    rscv = tiny.tile([128, 1], F32, tag="t1")
                    nc.vector.tensor_tensor(rscv[:], std[:], rgam, AO.mult)
                    gmw = tiny.tile([128, 1], F32, tag="t1")
                    nc.vector.tensor_scalar(gmw[:], gamv[:], 1.0 - alpha, None,
                                            AO.mult)
                    return th, gamv, rscv, gmw

                th1, gm1, _rsc1, gmw1 = stats_block(
                    g1, cpars[:, 0:1], cpars[:, 1:2], cpars[:, 4:5],
                    cpars[:, 6:7], alpha1)

                # ============ phase B + C: LIF1 + conv2 ============
                # one-time zeroing of spike-plane borders happens implicitly:
                # spike planes reuse the "pl" tag whose buffers held fully
                # DMA-written x planes with zero borders; interior writes and
                # the half-dup DMA never touch the borders afterwards.
                y2s = [None] * NPAIR
                for bp in range(2):
                    Pprev = [None] * NQ
                    for t in range(1, 5):
                        p = (t - 1) * 2 + bp
                        spl = []
                        for j in range(2):
                            sp = plpool.tile([128, PP], F16, tag="pl")
                            spl.append(sp)
                        spAr = spl[0].rearrange("p (r w) -> p r w", w=HP)
                        spBr = spl[1].rearrange("p (r w) -> p r w", w=HP)
                        for hq in range(NQ):
                            off = QL * hq
                            ysl = y1s[p][:, off:off + QL]
                            if t == 1:
                                qa = ysl
                            else:
                                q = hf.tile([128, QL], F32, tag="q2", bufs=3)
                                nc.gpsimd.tensor_tensor(q[:], ysl,
                                                        Pprev[hq][:], AO.add)
                                qa = q[:]
                            qa3 = qa.rearrange("p (r w) -> p r w", w=W)
                            dstA = spAr[0:64, 1 + 14 * hq:15 + 14 * hq,
                                        1:1 + W]
                            nc.vector.tensor_scalar(dstA, qa3[0:64],
                                                    th1[0:64], None, AO.is_ge)
                            dstB = spBr[64:128, 1 + 14 * hq:15 + 14 * hq,
                                        1:1 + W]
                            nc.vector.tensor_scalar(dstB, qa3[64:128],
                                                    th1[64:128], None,
                                                    AO.is_ge)
                            if t < 4:
                                sb = hf.tile([128, QL], F16, tag="sb", bufs=2)
                                nc.vector.tensor_scalar(sb[:], qa, th1[:],
                                                        None, AO.is_lt)
                                wv = hf.tile([128, QL], F32, tag="wv", bufs=2)
                                nc.scalar.activation(wv[:], qa, AF.Identity,
                                                     bias=gmw1[:],
                                                     scale=1.0 - alpha1)
                                Pn = hf.tile([128, QL], F32, tag="pp", bufs=5)
                                nc.gpsimd.tensor_tensor(Pn[:], wv[:], sb[:],
                                                        AO.mult)
                                Pprev[hq] = Pn
                        # duplicate spike halves: [sA;sA] and [sB;sB]
                        nc.scalar.dma_start(spl[0][64:128, :], spl[0][0:64, :])
                        nc.scalar.dma_start(spl[1][0:64, :], spl[1][64:128, :])
                        strip2 = yspool.tile([128, PIX], F32, tag=f"ys{p}",
                                             bufs=1)
                        y2s[p] = strip2
                        conv2_pair(spl[0], spl[1], strip2, p)

                # ---- stats2 allreduce
                cc2i = dramw.tile([128, 2], F32)
                cc2o = dramw.tile([128, 2], F32, addr_space="Shared")
                acc2 = tiny.tile([128, 2], F32, tag="acc")
                nc.vector.tensor_reduce(acc2[:, 0:1], sums2[:], AX.X, AO.add)
                nc.vector.tensor_reduce(acc2[:, 1:2], sums2q[:], AX.X, AO.add)
                nc.sync.dma_start(cc2i[:], acc2[:])
                if NO_CC:
                    nc.sync.dma_start(cc2o[:], cc2i[:])
                else:
                    nc.gpsimd.collective_compute(
                        "AllReduce", AO.add, ins=[cc2i[:]], outs=[cc2o[:]],
                        replica_groups=[list(range(NCORES))])
                g2 = tiny.tile([128, 2], F32, tag="acc")
                nc.sync.dma_start(g2[:], cc2o[:])
                th2, gm2, rsc2, gmw2 = stats_block(
                    g2, cpars[:, 2:3], cpars[:, 3:4], cpars[:, 5:6],
                    cpars[:, 7:8], alpha2)

                # ============ phase D: residual + LIF2 ============
                for bp in range(2):
                    Pprev2 = [None] * NQ
                    for t in range(1, 5):
                        p = (t - 1) * 2 + bp
                        iA = (t - 1) * 4 + bp * 2
                        for hq in range(NQ):
                            off = QL * hq
                            xs = hf.tile([128, QL], F32, tag="xs", bufs=3)
                            nc.sync.dma_start(
                                xs[:], xin[iA:iA + 2, :, off:off + QL])
                            # q = xs*rsc2 + y2 (+ P), built in place in xs
                            nc.scalar.activation(xs[:], xs[:], AF.Copy,
                                                 scale=rsc2[:])
                            nc.gpsimd.tensor_tensor(
                                xs[:], xs[:], y2s[p][:, off:off + QL], AO.add)
                            if t == 1:
                                q2v = xs[:]
                            else:
                                nc.vector.tensor_tensor(
                                    xs[:], xs[:], Pprev2[hq][:], AO.add)
                                q2v = xs[:]
                            ot = hf.tile([128, QL], F16, tag="ot", bufs=3)
                            nc.vector.tensor_scalar(ot[:], q2v, th2[:],
                                                    None, AO.is_ge)
                            nc.scalar.dma_start(
                                outp[iA:iA + 2, :, off:off + QL], ot[:])
                            if t < 4:
                                sb2 = hf.tile([128, QL], F16, tag="sb",
                                              bufs=2)
                                nc.vector.tensor_scalar(sb2[:], ot[:], 0.0,
                                                        None, AO.is_equal)
                                wv2 = hf.tile([128, QL], F32, tag="wv",
                                              bufs=2)
                                nc.scalar.activation(wv2[:], q2v, AF.Identity,
                                                     bias=gmw2[:],
                                                     scale=1.0 - alpha2)
                                Pn = hf.tile([128, QL], F32, tag="pp", bufs=5)
                                nc.gpsimd.tensor_tensor(Pn[:], wv2[:],
                                                        sb2[:], AO.mult)
                                Pprev2[hq] = Pn

    nc.compile()
    return nc, names


def _sigmoid(x):
    return 1.0 / (1.0 + np.exp(-float(x)))


def prepare(x, conv1_w, bn1_gamma, bn1_beta, lif1_w, conv2_w, bn2_gamma,
            bn2_beta, lif2_w):
    x = np.ascontiguousarray(np.asarray(x, np.float32))
    conv1_w = np.asarray(conv1_w, np.float32)
    conv2_w = np.asarray(conv2_w, np.float32)

    a1 = _sigmoid(np.asarray(lif1_w).reshape(-1)[0])
    a2 = _sigmoid(np.asarray(lif2_w).reshape(-1)[0])

    key = (round(a1, 12), round(a2, 12))
    if key not in _prog_cache:
        _prog_cache[key] = _build(a1, a2)
    nc, names = _prog_cache[key]

    # fp16 hi/lo split of x, padded planes (encoding only; exact split)
    xh = x.astype(np.float16)
    xl = (x - xh.astype(np.float32)).astype(np.float16)
    xpad = np.zeros((T, B, C, 2, HP, HP), np.float16)
    xpad[:, :, :, 0, 1:57, 1:57] = xh
    xpad[:, :, :, 1, 1:57, 1:57] = xl
    xpad = np.ascontiguousarray(xpad.transpose(0, 1, 3, 2, 4, 5))  # t,b,2,c,hp,hp

    # [xh ; xh shifted left one column (flat)] planes for the paired lo pass
    xhf = xpad[:, :, 0].reshape(T, B, C, PP)
    xbs = np.zeros((T, B, 2, C, PP), np.float16)
    xbs[:, :, 0] = xhf
    xbs[:, :, 1, :, :PP - 1] = xhf[:, :, :, 1:]

    w1h = conv1_w.astype(np.float16)
    w1l = (conv1_w - w1h.astype(np.float32)).astype(np.float16)
    w2h = conv2_w.astype(np.float16)
    w2l = (conv2_w - w2h.astype(np.float32)).astype(np.float16)

    def tapstack(wtop, wbot):
        out = np.zeros((128, 9 * 64), np.float16)
        for a in range(9):
            di, dj = a // 3, a % 3
            out[0:64, a * 64:(a + 1) * 64] = wtop[:, :, di, dj].T
            out[64:128, a * 64:(a + 1) * 64] = wbot[:, :, di, dj].T
        return out

    w1a_np = tapstack(w1h, w1h)
    w2a_np = tapstack(w2h, w2l)
    w1p_np = np.zeros((128, 3 * 64), np.float16)
    w1s_np = np.zeros((128, 3 * 64), np.float16)
    for di in range(3):
        w1p_np[0:64, di * 64:(di + 1) * 64] = w1l[:, :, di, 0].T
        w1p_np[64:128, di * 64:(di + 1) * 64] = w1l[:, :, di, 1].T
        w1s_np[0:64, di * 64:(di + 1) * 64] = w1l[:, :, di, 2].T

    def dup(v):
        v = np.asarray(v, np.float32).reshape(64)
        return np.concatenate([v, v])

    cpar_np = np.zeros((128, 8), np.float32)
    cpar_np[:, 0] = dup(bn1_gamma)
    cpar_np[:, 1] = dup(bn1_beta)
    cpar_np[:, 2] = dup(bn2_gamma)
    cpar_np[:, 3] = dup(bn2_beta)
    cpar_np[:, 4] = 1.0 / (a1 * dup(bn1_gamma))
    cpar_np[:, 5] = 1.0 / (a2 * dup(bn2_gamma))
    cpar_np[:, 6] = 1.0 / dup(bn1_gamma)
    cpar_np[:, 7] = 1.0 / dup(bn2_gamma)

    in_maps = []
    for k in range(NCORES):
        xta_np = np.ascontiguousarray(
            xpad[:, 4 * k:4 * k + 4].reshape(NIMG, 2, 64, PP))
        xtb_np = np.ascontiguousarray(
            xbs[:, 4 * k:4 * k + 4].reshape(NIMG, 2, 64, PP))
        xin_np = np.ascontiguousarray(
            x[:, 4 * k:4 * k + 4].reshape(NIMG, 64, PIX))
        in_maps.append({
            names['xta']: xta_np,
            names['xtb']: xtb_np,
            names['xin']: xin_np,
            names['w1a']: w1a_np,
            names['w1p']: w1p_np,
            names['w1s']: w1s_np,
            names['w2a']: w2a_np,
            names['cpar']: cpar_np,
        })

    return nc, names, in_maps


def kernel(**inputs):
    from concourse.bass_utils import run_bass_kernel_spmd
    nc, names, in_maps = prepare(**inputs)
    res = run_bass_kernel_spmd(nc, in_maps, core_ids=list(range(NCORES)))
    global LAST_RES, LAST_NAMES
    LAST_RES, LAST_NAMES = res, names
    out = np.empty((T, B, C, H, W), np.float32)
    for k in range(NCORES):
        o = res.results[k][names['outp']]
        out[:, 4 * k:4 * k + 4] = o.reshape(T, BL, C, H, W).astype(np.float32)
    return out


if __name__ == "__main__":
    rng = np.random.default_rng(0)
    xs = rng.standard_normal((T, B, C, H, W)).astype(np.float32)
    w1 = (rng.standard_normal((64, 64, 3, 3)) * 0.05).astype(np.float32)
    w2 = (rng.standard_normal((64, 64, 3, 3)) * 0.05).astype(np.float32)
    o = kernel(x=xs, conv1_w=w1, bn1_gamma=np.ones(64, np.float32),
               bn1_beta=np.zeros(64, np.float32),
               lif1_w=np.zeros(1, np.float32), conv2_w=w2,
               bn2_gamma=np.ones(64, np.float32),
               bn2_beta=np.zeros(64, np.float32),
               lif2_w=np.zeros(1, np.float32))
    print("ran:", o.shape, float(o.mean()))


# revision 32
# speedup vs baseline: 1.1708x; 1.0424x over previous
"""Trainium2 Bass kernel for nn_BasicBlock (spiking CNN block).

Sharding: data-parallel over batch B across 8 NeuronCores (4 batch x 4
timesteps = 16 images per core); BN batch stats via tiny AllReduce.

Per core:
- conv1: 15 matmuls per chunk-image (vs naive 18): hi pass per tap
  [W1hi;W1hi] x [xhi;xlo] (K=128, 9 taps), lo pass tap-paired
  [W1lo(di,0);W1lo(di,1)] x [xhi; xhi<<1col] (3) plus solo W1lo(di,2) x
  xhi (K=64, 3).  Exact fp16 hi/lo split arithmetic (~fp32 accuracy).
- BN stats (sum / sum-of-squares) accumulated during PSUM evacuation
  (ScalarE copy + Square, both with accum_out), all-reduced across cores.
- PLIF scans in "q-space" (conv-output units): BN scale/bias folded into
  per-channel threshold/constants.
- LIF1 spikes written by the compare op directly into the padded conv2
  plane interiors; the [s;s] duplication is one full-plane DMA.
- conv2 consumes exact 0/1 spikes in fp16: per tap [W2hi;W2lo] x [s1;s1]
  (K=128) gives both split terms in one matmul.
- Residual + LIF2 in quarter strips with image-pair-merged DMAs; x
  prefetched on the SP queue during conv2; output written as fp16.
"""
import sys
sys.path.insert(0, '/opt/trn_rl_repo')

import numpy as np

T, B, C, H, W = 4, 32, 64, 56, 56
NCORES = 8
BL = B // NCORES            # 4 local batch samples
NIMG = T * BL               # 16 images per core
HP = W + 2                  # 58
PP = HP * HP                # 3364 padded pixels
PIX = H * W                 # 3136
NCH = 7                     # conv chunks per image (8 rows each)
CHW = 8 * W                 # 448
NPAIR = 8                   # image pairs per core
EPS = 1e-5
NG = float((T * B) * PIX)   # 401408
QL = 14 * W                 # LIF quarter-strip length (784)
NQ = 4

_prog_cache = {}
NO_CC = False
TRACE = False
LAST_RES = None
LAST_NAMES = None


def _build(alpha1, alpha2):
    import concourse.mybir as mybir
    import concourse.tile as tile
    from concourse import bacc

    F32 = mybir.dt.float32
    F16 = mybir.dt.float16
    AO = mybir.AluOpType
    AF = mybir.ActivationFunctionType
    AX = mybir.AxisListType

    nc = bacc.Bacc(None, target_bir_lowering=False)
    names = {}

    with tile.TileContext(nc) as tc:
        with tc.tile_pool(name="dram", bufs=1, space="DRAM") as dram:
            xta = dram.tile([NIMG, 2, 64, PP], F16, kind="ExternalInput")
            xtb = dram.tile([NIMG, 2, 64, PP], F16, kind="ExternalInput")
            xin = dram.tile([NIMG, 64, PIX], F32, kind="ExternalInput")
            w1a = dram.tile([128, 9 * 64], F16, kind="ExternalInput")
            w1p = dram.tile([128, 3 * 64], F16, kind="ExternalInput")
            w1s = dram.tile([128, 3 * 64], F16, kind="ExternalInput")
            w2a = dram.tile([128, 9 * 64], F16, kind="ExternalInput")
            cpar = dram.tile([128, 8], F32, kind="ExternalInput")
            ident = dram.tile([128, 128], F32, kind="ExternalInput")
            outp = dram.tile([NIMG, 64, PIX], F16, kind="ExternalOutput")
            names.update(xta=xta.name, xtb=xtb.name, xin=xin.name,
                         w1a=w1a.name, w1p=w1p.name, w1s=w1s.name,
                         w2a=w2a.name, cpar=cpar.name, ident=ident.name,
                         outp=outp.name)

            with tc.tile_pool(name="dramw", bufs=1, space="DRAM") as dramw, \
                 tc.tile_pool(name="wsb", bufs=1) as wsb, \
                 tc.tile_pool(name="ys", bufs=8) as yspool, \
                 tc.tile_pool(name="plane", bufs=4) as plpool, \
                 tc.tile_pool(name="plb", bufs=2) as plbpool, \
                 tc.tile_pool(name="hfp", bufs=2) as hf, \
                 tc.tile_pool(name="tiny", bufs=16) as tiny, \
                 tc.tile_pool(name="ps", bufs=7, space="PSUM") as ps:

                # ---- static parameter loads (Act queue; planes go on SP)
                w1as = wsb.tile([128, 9 * 64], F16, tag="w1a")
                nc.scalar.dma_start(w1as[:], w1a[:])
                w1ps = wsb.tile([128, 3 * 64], F16, tag="w1p")
                nc.scalar.dma_start(w1ps[:], w1p[:])
                w1ss = wsb.tile([128, 3 * 64], F16, tag="w1s")
                nc.scalar.dma_start(w1ss[:], w1s[:])
                w2as = wsb.tile([128, 9 * 64], F16, tag="w2a")
                nc.scalar.dma_start(w2as[:], w2a[:])
                cpars = wsb.tile([128, 8], F32, tag="cpar")
                nc.scalar.dma_start(cpars[:], cpar[:])
                idents = wsb.tile([128, 128], F32, tag="ident")
                nc.scalar.dma_start(idents[:], ident[:])
                sums1 = wsb.tile([128, 56], F32, tag="sums1")
                sums1q = wsb.tile([128, 56], F32, tag="sums1q")
                sums2 = wsb.tile([128, 56], F32, tag="sums2")
                sums2q = wsb.tile([128, 56], F32, tag="sums2q")
                # preload the activation-function table set containing Sqrt
                # during startup so the stats blocks don't pay the 1.3us
                # table swap on the critical path
                sqwarm = tiny.tile([128, 1], F32, tag="t1")
                nc.vector.memset(sqwarm[:], 1.0)
                sqw2 = tiny.tile([128, 1], F32, tag="t1")
                nc.scalar.activation(sqw2[:], sqwarm[:], AF.Sqrt)

                def evac(pts, dst, sums_t, sumsq_t, col):
                    nc.scalar.activation(dst, pts[:], AF.Copy,
                                         accum_out=sums_t[:, col:col + 1])
                    nc.scalar.activation(pts[:], pts[:], AF.Square,
                                         accum_out=sumsq_t[:, col:col + 1])

                def conv1_pair(plA, plB, pbA, pbB, dst_strip, pcol):
                    """Conv1 for one image pair: per chunk 9 hi-pass K=128
                    matmuls ([W1hi;W1hi] x [xh;xl]) + 3 tap-paired lo
                    ([W1lo(di,0);W1lo(di,1)] x [xh;xh<<1]) + 3 solo K=64
                    (W1lo(di,2) x xh).  Waves of 4+3 chunks so evacuations
                    overlap the next wave."""
                    plAr = plA.rearrange("p (r w) -> p r w", w=HP)
                    plBr = plB.rearrange("p (r w) -> p r w", w=HP)
                    pbAr = pbA.rearrange("p (r w) -> p r w", w=HP)
                    pbBr = pbB.rearrange("p (r w) -> p r w", w=HP)
                    for wave in (range(0, 4), range(4, 7)):
                        pts = {}
                        for cth in wave:
                            pts[cth] = ps.tile([128, CHW], F32, tag="ps",
                                               bufs=7, name=f"psum{cth}")
                        for j in range(2):
                            plr = (plAr, plBr)[j]
                            pbr = (pbAr, pbBr)[j]
                            for a in range(9):
                                di, dj = a // 3, a % 3
                                for cth in wave:
                                    r0 = 8 * cth + di
                                    out = pts[cth][64 * j:64 * (j + 1), :] \
                                        .rearrange("p (r w) -> p r w", r=8)
                                    nc.tensor.matmul(
                                        out, w1as[:, a * 64:(a + 1) * 64],
                                        plr[:, r0:r0 + 8, dj:dj + W],
                                        start=(a == 0), stop=False,
                                        tile_position=(0, 64 * j),
                                        skip_group_check=True)
                            for di in range(3):
                                for cth in wave:
                                    r0 = 8 * cth + di
                                    out = pts[cth][64 * j:64 * (j + 1), :] \
                                        .rearrange("p (r w) -> p r w", r=8)
                                    nc.tensor.matmul(
                                        out, w1ps[:, di * 64:(di + 1) * 64],
                                        pbr[:, r0:r0 + 8, 0:W],
                                        start=False, stop=False,
                                        tile_position=(0, 64 * j),
                                        skip_group_check=True)
                            for di in range(3):
                                for cth in wave:
                                    r0 = 8 * cth + di
                                    out = pts[cth][64 * j:64 * (j + 1), :] \
                                        .rearrange("p (r w) -> p r w", r=8)
                                    nc.tensor.matmul(
                                        out, w1ss[0:64, di * 64:(di + 1) * 64],
                                        plr[0:64, r0:r0 + 8, 2:2 + W],
                                        start=False, stop=(di == 2),
                                        tile_position=(0, 64 * j),
                                        skip_group_check=True)
                        for cth in wave:
                            evac(pts[cth],
                                 dst_strip[:, CHW * cth:CHW * (cth + 1)],
                                 sums1, sums1q, pcol * 7 + cth)

                def conv2_pair(plA, plB, dst_strip, pcol):
                    plAr = plA.rearrange("p (r w) -> p r w", w=HP)
                    plBr = plB.rearrange("p (r w) -> p r w", w=HP)
                    for wave in (range(0, 4), range(4, 7)):
                        pts = {}
                        for cth in wave:
                            pts[cth] = ps.tile([128, CHW], F32, tag="ps",
                                               bufs=7, name=f"psum{cth}")
                        for j in range(2):
                            plr = (plAr, plBr)[j]
                            for a in range(9):
                                di, dj = a // 3, a % 3
                                for cth in wave:
                                    r0 = 8 * cth + di
                                    out = pts[cth][64 * j:64 * (j + 1), :] \
                                        .rearrange("p (r w) -> p r w", r=8)
                                    nc.tensor.matmul(
                                        out, w2as[:, a * 64:(a + 1) * 64],
                                        plr[:, r0:r0 + 8, dj:dj + W],
                                        start=(a == 0), stop=(a == 8),
                                        tile_position=(0, 64 * j),
                                        skip_group_check=True)
                        for cth in wave:
                            evac(pts[cth],
                                 dst_strip[:, CHW * cth:CHW * (cth + 1)],
                                 sums2, sums2q, pcol * 7 + cth)

                # ================= phase A: conv1 =================
                y1s = []
                for p in range(NPAIR):
                    tt_, bp = p // 2, p % 2
                    iA = tt_ * 4 + bp * 2
                    tas, pbs = [], []
                    for j in range(2):
                        i = iA + j
                        ta = plpool.tile([128, PP], F16, tag="pl")
                        nc.sync.dma_start(ta[:], xta[i])
                        tas.append(ta)
                        pb = plbpool.tile([128, PP], F16, tag="plb")
                        nc.scalar.dma_start(pb[:], xtb[i])
                        pbs.append(pb)
                    strip = yspool.tile([128, PIX], F32, tag=f"ys{p}",
                                        bufs=1)
                    y1s.append(strip)
                    conv1_pair(tas[0], tas[1], pbs[0], pbs[1], strip, p)

                # ---- stats1 allreduce
                cc1i = dramw.tile([128, 2], F32)
                cc1o = dramw.tile([128, 2], F32, addr_space="Shared")
                acc1 = tiny.tile([128, 2], F32, tag="acc")
                nc.vector.tensor_reduce(acc1[:, 0:1], sums1[:], AX.X, AO.add)
                nc.vector.tensor_reduce(acc1[:, 1:2], sums1q[:], AX.X, AO.add)
                nc.sync.dma_start(cc1i[:], acc1[:])
                if NO_CC:
                    nc.sync.dma_start(cc1o[:], cc1i[:])
                else:
                    nc.gpsimd.collective_compute(
                        "AllReduce", AO.add, ins=[cc1i[:]], outs=[cc1o[:]],
                        replica_groups=[list(range(NCORES))])
                g1 = tiny.tile([128, 2], F32, tag="acc")
                nc.sync.dma_start(g1[:], cc1o[:])

                def stats_block(g, gamma, beta, rga, rgam, alpha):
                    # cross-half channel sum on the (idle) PE: out[c] =
                    # g[c%64] + g[c%64+64] via a stacked-identity fp32 matmul
                    totp = ps.tile([128, 2], F32, tag="pstot", bufs=1,
                                   name="pstot")
                    nc.tensor.matmul(totp[:], idents[:], g[:],
                                     start=True, stop=True,
                                     skip_group_check=True)
                    tot = totp
                    mm = tiny.tile([128, 2], F32, tag="acc")
                    nc.vector.tensor_scalar(mm[:], tot[:], 1.0 / NG, None,
                                            AO.mult)
                    mean = mm[:, 0:1]
                    m2 = tiny.tile([128, 1], F32, tag="t1")
                    nc.vector.tensor_tensor(m2[:], mean, mean, AO.mult)
                    var = tiny.tile([128, 1], F32, tag="t1")
                    nc.vector.tensor_tensor(var[:], mm[:, 1:2], m2[:],
                                            AO.subtract)
                    epst = tiny.tile([128, 1], F32, tag="t1")
                    nc.vector.memset(epst[:], EPS)
                    std = tiny.tile([128, 1], F32, tag="t1")
                    nc.scalar.activation(std[:], var[:], AF.Sqrt, bias=epst[:])
                    rstd = tiny.tile([128, 1], F32, tag="t1")
                    nc.vector.reciprocal(rstd[:], std[:])
                    rscv = tiny.tile([128, 1], F32, tag="t1")
                    nc.vector.tensor_tensor(rscv[:], std[:], rgam, AO.mult)
                    sc = tiny.tile([128, 1], F32, tag="t1")
                    nc.vector.tensor_tensor(sc[:], gamma, rstd[:], AO.mult)
                    nmsc = tiny.tile([128, 1], F32, tag="t1")
                    nc.vector.scalar_tensor_tensor(nmsc[:], mean, -1.0, sc[:],
                                                   AO.mult, AO.mult)
                    bi = tiny.tile([128, 1], F32, tag="t1")
                    nc.vector.tensor_tensor(bi[:], beta, nmsc[:], AO.add)
                    stdrg = tiny.tile([128, 1], F32, tag="t1")
                    nc.vector.tensor_tensor(stdrg[:], std[:], rga, AO.mult)
                    nbst = tiny.tile([128, 1], F32, tag="t1")
                    nc.vector.scalar_tensor_tensor(nbst[:], bi[:], -alpha,
                                                   stdrg[:], AO.mult, AO.mult)
                    th = tiny.tile([128, 1], F32, tag="t1")
                    nc.vector.tensor_tensor(th[:], stdrg[:], nbst[:], AO.add)
                    bstd = tiny.tile([128, 1], F32, tag="t1")
                    nc.vector.tensor_tensor(bstd[:], bi[:], std[:], AO.mult)
                    gamv = tiny.tile([128, 1], F32, tag="t1")
                    nc.vector.tensor_tensor(gamv[:], bstd[:], rgam, AO.mult)
                    gmw = tiny.tile([128, 1], F32, tag="t1")
                    nc.vector.tensor_scalar(gmw[:], gamv[:], 1.0 - alpha, None,
                                            AO.mult)
                    return th, gamv, rscv, gmw

                th1, gm1, _rsc1, gmw1 = stats_block(
                    g1, cpars[:, 0:1], cpars[:, 1:2], cpars[:, 4:5],
                    cpars[:, 6:7], alpha1)

                # ============ phase B + C: LIF1 + conv2 ============
                # one-time zeroing of spike-plane borders happens implicitly:
                # spike planes reuse the "pl" tag whose buffers held fully
                # DMA-written x planes with zero borders; interior writes and
                # the half-dup DMA never touch the borders afterwards.
                y2s = [None] * NPAIR
                for bp in range(2):
                    Pprev = [None] * NQ
                    for t in range(1, 5):
                        p = (t - 1) * 2 + bp
                        spl = []
                        for j in range(2):
                            sp = plpool.tile([128, PP], F16, tag="pl")
                            spl.append(sp)
                        spAr = spl[0].rearrange("p (r w) -> p r w", w=HP)
                        spBr = spl[1].rearrange("p (r w) -> p r w", w=HP)
                        qas = []
                        for hq in range(NQ):
                            off = QL * hq
                            ysl = y1s[p][:, off:off + QL]
                            if t == 1:
                                qa = ysl
                            else:
                                q = hf.tile([128, QL], F32, tag="q2", bufs=4)
                                nc.gpsimd.tensor_tensor(q[:], ysl,
                                                        Pprev[hq][:], AO.add)
                                qa = q[:]
                            qas.append(qa)
                            qa3 = qa.rearrange("p (r w) -> p r w", w=W)
                            dstA = spAr[0:64, 1 + 14 * hq:15 + 14 * hq,
                                        1:1 + W]
                            nc.vector.tensor_scalar(dstA, qa3[0:64],
                                                    th1[0:64], None, AO.is_ge)
                        # image A plane complete first so j=0 matmuls start
                        nc.scalar.dma_start(spl[0][64:128, :], spl[0][0:64, :])
                        for hq in range(NQ):
                            qa3 = qas[hq].rearrange("p (r w) -> p r w", w=W)
                            dstB = spBr[64:128, 1 + 14 * hq:15 + 14 * hq,
                                        1:1 + W]
                            nc.vector.tensor_scalar(dstB, qa3[64:128],
                                                    th1[64:128], None,
                                                    AO.is_ge)
                        nc.scalar.dma_start(spl[1][0:64, :], spl[1][64:128, :])
                        if t < 4:
                            for hq in range(NQ):
                                qa = qas[hq]
                                sb = hf.tile([128, QL], F16, tag="sb", bufs=2)
                                nc.vector.tensor_scalar(sb[:], qa, th1[:],
                                                        None, AO.is_lt)
                                wv = hf.tile([128, QL], F32, tag="wv", bufs=2)
                                nc.scalar.activation(wv[:], qa, AF.Identity,
                                                     bias=gmw1[:],
                                                     scale=1.0 - alpha1)
                                Pn = hf.tile([128, QL], F32, tag="pp", bufs=5)
                                nc.gpsimd.tensor_tensor(Pn[:], wv[:], sb[:],
                                                        AO.mult)
                                Pprev[hq] = Pn
                        strip2 = yspool.tile([128, PIX], F32, tag=f"ys{p}",
                                             bufs=1)
                        y2s[p] = strip2
                        conv2_pair(spl[0], spl[1], strip2, p)

                # ---- stats2 allreduce
                cc2i = dramw.tile([128, 2], F32)
                cc2o = dramw.tile([128, 2], F32, addr_space="Shared")
                acc2 = tiny.tile([128, 2], F32, tag="acc")
                nc.vector.tensor_reduce(acc2[:, 0:1], sums2[:], AX.X, AO.add)
                nc.vector.tensor_reduce(acc2[:, 1:2], sums2q[:], AX.X, AO.add)
                nc.sync.dma_start(cc2i[:], acc2[:])
                if NO_CC:
                    nc.sync.dma_start(cc2o[:], cc2i[:])
                else:
                    nc.gpsimd.collective_compute(
                        "AllReduce", AO.add, ins=[cc2i[:]], outs=[cc2o[:]],
                        replica_groups=[list(range(NCORES))])
                g2 = tiny.tile([128, 2], F32, tag="acc")
                nc.sync.dma_start(g2[:], cc2o[:])
                th2, gm2, rsc2, gmw2 = stats_block(
                    g2, cpars[:, 2:3], cpars[:, 3:4], cpars[:, 5:6],
                    cpars[:, 7:8], alpha2)

                # ============ phase D: residual + LIF2 ============
                # flat t-outer iteration list, software-pipelined: xs loads
                # are emitted K iterations ahead on SP, and out-DMAs also go
                # on SP *behind* the prefetched loads, so their wait on `ot`
                # never blocks loads or any compute queue.
                Pprev2 = {0: [None] * NQ, 1: [None] * NQ}
                iters = [(t, bp, hq) for t in range(1, 5)
                         for bp in range(2) for hq in range(NQ)]
                KPF = 6
                xstiles = {}

                def issue_load(idx):
                    t, bp, hq = iters[idx]
                    iA = (t - 1) * 4 + bp * 2
                    off = QL * hq
                    xs = hf.tile([128, QL], F32, tag="xs", bufs=8)
                    nc.sync.dma_start(xs[:],
                                      xin[iA:iA + 2, :, off:off + QL])
                    xstiles[idx] = xs

                for idx in range(KPF):
                    issue_load(idx)
                for idx, (t, bp, hq) in enumerate(iters):
                    if idx + KPF < len(iters):
                        issue_load(idx + KPF)
                    p = (t - 1) * 2 + bp
                    iA = (t - 1) * 4 + bp * 2
                    off = QL * hq
                    xs = xstiles.pop(idx)
                    # q = xs*rsc2 + y2 (+ P), built in place in xs
                    nc.scalar.activation(xs[:], xs[:], AF.Copy,
                                         scale=rsc2[:])
                    if t < 4:
                        nc.vector.tensor_tensor(
                            xs[:], xs[:], y2s[p][:, off:off + QL], AO.add)
                    else:
                        nc.gpsimd.tensor_tensor(
                            xs[:], xs[:], y2s[p][:, off:off + QL], AO.add)
                    if t > 1:
                        eng = nc.vector if (t == 4 or hq % 2) else nc.gpsimd
                        eng.tensor_tensor(xs[:], xs[:],
                                          Pprev2[bp][hq][:], AO.add)
                    q2v = xs[:]
                    ot = hf.tile([128, QL], F16, tag="ot", bufs=2)
                    nc.vector.tensor_scalar(ot[:], q2v, th2[:],
                                            None, AO.is_ge)
                    if t < 4:
                        sb2 = hf.tile([128, QL], F16, tag="sb", bufs=2)
                        nc.vector.tensor_scalar(sb2[:], ot[:], 0.0,
                                                None, AO.is_equal)
                        wv2 = hf.tile([128, QL], F32, tag="wv", bufs=2)
                        nc.scalar.activation(wv2[:], q2v, AF.Identity,
                                             bias=gmw2[:],
                                             scale=1.0 - alpha2)
                        ptag = ("pp", 5) if bp == 0 else ("q2", 4)
                        Pn = hf.tile([128, QL], F32, tag=ptag[0],
                                     bufs=ptag[1])
                        nc.gpsimd.tensor_tensor(Pn[:], wv2[:], sb2[:],
                                                AO.mult)
                        Pprev2[bp][hq] = Pn
                    oeng = nc.scalar if idx % 2 else nc.sync
                    oeng.dma_start(outp[iA:iA + 2, :, off:off + QL],
                                   ot[:])

    nc.compile()
    return nc, names


def _sigmoid(x):
    return 1.0 / (1.0 + np.exp(-float(x)))


def prepare(x, conv1_w, bn1_gamma, bn1_beta, lif1_w, conv2_w, bn2_gamma,
            bn2_beta, lif2_w):
    x = np.ascontiguousarray(np.asarray(x, np.float32))
    conv1_w = np.asarray(conv1_w, np.float32)
    conv2_w = np.asarray(conv2_w, np.float32)

    a1 = _sigmoid(np.asarray(lif1_w).reshape(-1)[0])
    a2 = _sigmoid(np.asarray(lif2_w).reshape(-1)[0])

    key = (round(a1, 12), round(a2, 12))
    if key not in _prog_cache:
        _prog_cache[key] = _build(a1, a2)
    nc, names = _prog_cache[key]

    # fp16 hi/lo split of x, padded planes (encoding only; exact split)
    xh = x.astype(np.float16)
    xl = (x - xh.astype(np.float32)).astype(np.float16)
    xpad = np.zeros((T, B, C, 2, HP, HP), np.float16)
    xpad[:, :, :, 0, 1:57, 1:57] = xh
    xpad[:, :, :, 1, 1:57, 1:57] = xl
    xpad = np.ascontiguousarray(xpad.transpose(0, 1, 3, 2, 4, 5))  # t,b,2,c,hp,hp

    # [xh ; xh shifted left one column (flat)] planes for the paired lo pass
    xhf = xpad[:, :, 0].reshape(T, B, C, PP)
    xbs = np.zeros((T, B, 2, C, PP), np.float16)
    xbs[:, :, 0] = xhf
    xbs[:, :, 1, :, :PP - 1] = xhf[:, :, :, 1:]

    w1h = conv1_w.astype(np.float16)
    w1l = (conv1_w - w1h.astype(np.float32)).astype(np.float16)
    w2h = conv2_w.astype(np.float16)
    w2l = (conv2_w - w2h.astype(np.float32)).astype(np.float16)

    def tapstack(wtop, wbot):
        out = np.zeros((128, 9 * 64), np.float16)
        for a in range(9):
            di, dj = a // 3, a % 3
            out[0:64, a * 64:(a + 1) * 64] = wtop[:, :, di, dj].T
            out[64:128, a * 64:(a + 1) * 64] = wbot[:, :, di, dj].T
        return out

    w1a_np = tapstack(w1h, w1h)
    w2a_np = tapstack(w2h, w2l)
    w1p_np = np.zeros((128, 3 * 64), np.float16)
    w1s_np = np.zeros((128, 3 * 64), np.float16)
    for di in range(3):
        w1p_np[0:64, di * 64:(di + 1) * 64] = w1l[:, :, di, 0].T
        w1p_np[64:128, di * 64:(di + 1) * 64] = w1l[:, :, di, 1].T
        w1s_np[0:64, di * 64:(di + 1) * 64] = w1l[:, :, di, 2].T

    def dup(v):
        v = np.asarray(v, np.float32).reshape(64)
        return np.concatenate([v, v])

    cpar_np = np.zeros((128, 8), np.float32)
    cpar_np[:, 0] = dup(bn1_gamma)
    cpar_np[:, 1] = dup(bn1_beta)
    cpar_np[:, 2] = dup(bn2_gamma)
    cpar_np[:, 3] = dup(bn2_beta)
    cpar_np[:, 4] = 1.0 / (a1 * dup(bn1_gamma))
    cpar_np[:, 5] = 1.0 / (a2 * dup(bn2_gamma))
    cpar_np[:, 6] = 1.0 / dup(bn1_gamma)
    cpar_np[:, 7] = 1.0 / dup(bn2_gamma)

    kk, mm_ = np.meshgrid(np.arange(128), np.arange(128), indexing='ij')
    ident_np = (kk % 64 == mm_ % 64).astype(np.float32)

    in_maps = []
    for k in range(NCORES):
        xta_np = np.ascontiguousarray(
            xpad[:, 4 * k:4 * k + 4].reshape(NIMG, 2, 64, PP))
        xtb_np = np.ascontiguousarray(
            xbs[:, 4 * k:4 * k + 4].reshape(NIMG, 2, 64, PP))
        xin_np = np.ascontiguousarray(
            x[:, 4 * k:4 * k + 4].reshape(NIMG, 64, PIX))
        in_maps.append({
            names['xta']: xta_np,
            names['xtb']: xtb_np,
            names['xin']: xin_np,
            names['w1a']: w1a_np,
            names['w1p']: w1p_np,
            names['w1s']: w1s_np,
            names['w2a']: w2a_np,
            names['cpar']: cpar_np,
            names['ident']: ident_np,
        })

    return nc, names, in_maps


def kernel(**inputs):
    from concourse.bass_utils import run_bass_kernel_spmd
    nc, names, in_maps = prepare(**inputs)
    res = run_bass_kernel_spmd(nc, in_maps, core_ids=list(range(NCORES)))
    global LAST_RES, LAST_NAMES
    LAST_RES, LAST_NAMES = res, names
    out = np.empty((T, B, C, H, W), np.float32)
    for k in range(NCORES):
        o = res.results[k][names['outp']]
        out[:, 4 * k:4 * k + 4] = o.reshape(T, BL, C, H, W).astype(np.float32)
    return out


if __name__ == "__main__":
    rng = np.random.default_rng(0)
    xs = rng.standard_normal((T, B, C, H, W)).astype(np.float32)
    w1 = (rng.standard_normal((64, 64, 3, 3)) * 0.05).astype(np.float32)
    w2 = (rng.standard_normal((64, 64, 3, 3)) * 0.05).astype(np.float32)
    o = kernel(x=xs, conv1_w=w1, bn1_gamma=np.ones(64, np.float32),
               bn1_beta=np.zeros(64, np.float32),
               lif1_w=np.zeros(1, np.float32), conv2_w=w2,
               bn2_gamma=np.ones(64, np.float32),
               bn2_beta=np.zeros(64, np.float32),
               lif2_w=np.zeros(1, np.float32))
    print("ran:", o.shape, float(o.mean()))


# revision 37
# speedup vs baseline: 1.1740x; 1.0028x over previous
"""Trainium2 Bass kernel for nn_BasicBlock (spiking CNN block).

Sharding: data-parallel over batch B across 8 NeuronCores (4 batch x 4
timesteps = 16 images per core); BN batch stats via tiny AllReduce.

Per core:
- conv1: 15 matmuls per chunk-image (vs naive 18): hi pass per tap
  [W1hi;W1hi] x [xhi;xlo] (K=128, 9 taps), lo pass tap-paired
  [W1lo(di,0);W1lo(di,1)] x [xhi; xhi<<1col] (3) plus solo W1lo(di,2) x
  xhi (K=64, 3).  Exact fp16 hi/lo split arithmetic (~fp32 accuracy).
- BN stats (sum / sum-of-squares) accumulated during PSUM evacuation
  (ScalarE copy + Square, both with accum_out), all-reduced across cores.
- PLIF scans in "q-space" (conv-output units): BN scale/bias folded into
  per-channel threshold/constants.
- LIF1 spikes written by the compare op directly into the padded conv2
  plane interiors; the [s;s] duplication is one full-plane DMA.
- conv2 consumes exact 0/1 spikes in fp16: per tap [W2hi;W2lo] x [s1;s1]
  (K=128) gives both split terms in one matmul.
- Residual + LIF2 in quarter strips with image-pair-merged DMAs; x
  prefetched on the SP queue during conv2; output written as fp16.
"""
import sys
sys.path.insert(0, '/opt/trn_rl_repo')

import numpy as np

T, B, C, H, W = 4, 32, 64, 56, 56
NCORES = 8
BL = B // NCORES            # 4 local batch samples
NIMG = T * BL               # 16 images per core
HP = W + 2                  # 58
PP = HP * HP                # 3364 padded pixels
PIX = H * W                 # 3136
NCH = 7                     # conv chunks per image (8 rows each)
CHW = 8 * W                 # 448
NPAIR = 8                   # image pairs per core
EPS = 1e-5
NG = float((T * B) * PIX)   # 401408
QL = 14 * W                 # LIF quarter-strip length (784)
NQ = 4

_prog_cache = {}
NO_CC = False
TRACE = False
LAST_RES = None
LAST_NAMES = None


def _build(alpha1, alpha2):
    import concourse.mybir as mybir
    import concourse.tile as tile
    from concourse import bacc

    F32 = mybir.dt.float32
    F16 = mybir.dt.float16
    AO = mybir.AluOpType
    AF = mybir.ActivationFunctionType
    AX = mybir.AxisListType

    nc = bacc.Bacc(None, target_bir_lowering=False)
    names = {}

    with tile.TileContext(nc) as tc:
        with tc.tile_pool(name="dram", bufs=1, space="DRAM") as dram:
            xta = dram.tile([NIMG, 2, 64, PP], F16, kind="ExternalInput")
            xtb = dram.tile([NIMG, 2, 64, PP], F16, kind="ExternalInput")
            xin = dram.tile([NIMG, 64, PIX], F32, kind="ExternalInput")
            w1a = dram.tile([128, 9 * 64], F16, kind="ExternalInput")
            w1p = dram.tile([128, 3 * 64], F16, kind="ExternalInput")
            w1s = dram.tile([128, 3 * 64], F16, kind="ExternalInput")
            w2a = dram.tile([128, 9 * 64], F16, kind="ExternalInput")
            cpar = dram.tile([128, 8], F32, kind="ExternalInput")
            ident = dram.tile([128, 128], F32, kind="ExternalInput")
            outp = dram.tile([NIMG, 64, PIX], F16, kind="ExternalOutput")
            names.update(xta=xta.name, xtb=xtb.name, xin=xin.name,
                         w1a=w1a.name, w1p=w1p.name, w1s=w1s.name,
                         w2a=w2a.name, cpar=cpar.name, ident=ident.name,
                         outp=outp.name)

            with tc.tile_pool(name="dramw", bufs=1, space="DRAM") as dramw, \
                 tc.tile_pool(name="wsb", bufs=1) as wsb, \
                 tc.tile_pool(name="ys", bufs=8) as yspool, \
                 tc.tile_pool(name="plane", bufs=4) as plpool, \
                 tc.tile_pool(name="plb", bufs=2) as plbpool, \
                 tc.tile_pool(name="hfp", bufs=2) as hf, \
                 tc.tile_pool(name="tiny", bufs=16) as tiny, \
                 tc.tile_pool(name="ps", bufs=7, space="PSUM") as ps:

                # ---- static parameter loads (Act queue; planes go on SP)
                w1as = wsb.tile([128, 9 * 64], F16, tag="w1a")
                nc.scalar.dma_start(w1as[:], w1a[:])
                w1ps = wsb.tile([128, 3 * 64], F16, tag="w1p")
                nc.scalar.dma_start(w1ps[:], w1p[:])
                w1ss = wsb.tile([128, 3 * 64], F16, tag="w1s")
                nc.scalar.dma_start(w1ss[:], w1s[:])
                w2as = wsb.tile([128, 9 * 64], F16, tag="w2a")
                nc.scalar.dma_start(w2as[:], w2a[:])
                cpars = wsb.tile([128, 8], F32, tag="cpar")
                nc.scalar.dma_start(cpars[:], cpar[:])
                idents = wsb.tile([128, 128], F32, tag="ident")
                nc.scalar.dma_start(idents[:], ident[:])
                sums1 = wsb.tile([128, 56], F32, tag="sums1")
                sums1q = wsb.tile([128, 56], F32, tag="sums1q")
                sums2 = wsb.tile([128, 56], F32, tag="sums2")
                sums2q = wsb.tile([128, 56], F32, tag="sums2q")
                # preload the activation-function table set containing Sqrt
                # during startup so the stats blocks don't pay the 1.3us
                # table swap on the critical path
                sqwarm = tiny.tile([128, 1], F32, tag="t1")
                nc.vector.memset(sqwarm[:], 1.0)
                sqw2 = tiny.tile([128, 1], F32, tag="t1")
                nc.scalar.activation(sqw2[:], sqwarm[:], AF.Sqrt)

                def evac(pts, dst, sums_t, sumsq_t, col):
                    nc.scalar.activation(dst, pts[:], AF.Copy,
                                         accum_out=sums_t[:, col:col + 1])
                    nc.scalar.activation(pts[:], pts[:], AF.Square,
                                         accum_out=sumsq_t[:, col:col + 1])

                def conv1_pair(plA, plB, pbA, pbB, dst_strip, pcol):
                    """Conv1 for one image pair: per chunk 9 hi-pass K=128
                    matmuls ([W1hi;W1hi] x [xh;xl]) + 3 tap-paired lo
                    ([W1lo(di,0);W1lo(di,1)] x [xh;xh<<1]) + 3 solo K=64
                    (W1lo(di,2) x xh).  Waves of 4+3 chunks so evacuations
                    overlap the next wave."""
                    plAr = plA.rearrange("p (r w) -> p r w", w=HP)
                    plBr = plB.rearrange("p (r w) -> p r w", w=HP)
                    pbAr = pbA.rearrange("p (r w) -> p r w", w=HP)
                    pbBr = pbB.rearrange("p (r w) -> p r w", w=HP)
                    for wave in (range(0, 4), range(4, 7)):
                        pts = {}
                        for cth in wave:
                            pts[cth] = ps.tile([128, CHW], F32, tag="ps",
                                               bufs=7, name=f"psum{cth}")
                        for j in range(2):
                            plr = (plAr, plBr)[j]
                            pbr = (pbAr, pbBr)[j]
                            for a in range(9):
                                di, dj = a // 3, a % 3
                                for cth in wave:
                                    r0 = 8 * cth + di
                                    out = pts[cth][64 * j:64 * (j + 1), :] \
                                        .rearrange("p (r w) -> p r w", r=8)
                                    nc.tensor.matmul(
                                        out, w1as[:, a * 64:(a + 1) * 64],
                                        plr[:, r0:r0 + 8, dj:dj + W],
                                        start=(a == 0), stop=False,
                                        tile_position=(0, 64 * j),
                                        skip_group_check=True)
                            for di in range(3):
                                for cth in wave:
                                    r0 = 8 * cth + di
                                    out = pts[cth][64 * j:64 * (j + 1), :] \
                                        .rearrange("p (r w) -> p r w", r=8)
                                    nc.tensor.matmul(
                                        out, w1ps[:, di * 64:(di + 1) * 64],
                                        pbr[:, r0:r0 + 8, 0:W],
                                        start=False, stop=False,
                                        tile_position=(0, 64 * j),
                                        skip_group_check=True)
                            for di in range(3):
                                for cth in wave:
                                    r0 = 8 * cth + di
                                    out = pts[cth][64 * j:64 * (j + 1), :] \
                                        .rearrange("p (r w) -> p r w", r=8)
                                    nc.tensor.matmul(
                                        out, w1ss[0:64, di * 64:(di + 1) * 64],
                                        plr[0:64, r0:r0 + 8, 2:2 + W],
                                        start=False, stop=(di == 2),
                                        tile_position=(0, 64 * j),
                                        skip_group_check=True)
                        for cth in wave:
                            evac(pts[cth],
                                 dst_strip[:, CHW * cth:CHW * (cth + 1)],
                                 sums1, sums1q, pcol * 7 + cth)

                def conv2_pair(plA, plB, dst_strip, pcol):
                    plAr = plA.rearrange("p (r w) -> p r w", w=HP)
                    plBr = plB.rearrange("p (r w) -> p r w", w=HP)
                    for wave in (range(0, 4), range(4, 7)):
                        pts = {}
                        for cth in wave:
                            pts[cth] = ps.tile([128, CHW], F32, tag="ps",
                                               bufs=7, name=f"psum{cth}")
                        for j in range(2):
                            plr = (plAr, plBr)[j]
                            for a in range(9):
                                di, dj = a // 3, a % 3
                                for cth in wave:
                                    r0 = 8 * cth + di
                                    out = pts[cth][64 * j:64 * (j + 1), :] \
                                        .rearrange("p (r w) -> p r w", r=8)
                                    nc.tensor.matmul(
                                        out, w2as[:, a * 64:(a + 1) * 64],
                                        plr[:, r0:r0 + 8, dj:dj + W],
                                        start=(a == 0), stop=(a == 8),
                                        tile_position=(0, 64 * j),
                                        skip_group_check=True)
                        for cth in wave:
                            evac(pts[cth],
                                 dst_strip[:, CHW * cth:CHW * (cth + 1)],
                                 sums2, sums2q, pcol * 7 + cth)

                # ================= phase A: conv1 =================
                y1s = []
                for p in range(NPAIR):
                    tt_, bp = p // 2, p % 2
                    iA = tt_ * 4 + bp * 2
                    tas, pbs = [], []
                    for j in range(2):
                        i = iA + j
                        ta = plpool.tile([128, PP], F16, tag="pl")
                        nc.sync.dma_start(ta[:], xta[i])
                        tas.append(ta)
                        pb = plbpool.tile([128, PP], F16, tag="plb")
                        nc.scalar.dma_start(pb[:], xtb[i])
                        pbs.append(pb)
                    strip = yspool.tile([128, PIX], F32, tag=f"ys{p}",
                                        bufs=1)
                    y1s.append(strip)
                    conv1_pair(tas[0], tas[1], pbs[0], pbs[1], strip, p)

                # ---- stats1 allreduce
                cc1i = dramw.tile([128, 2], F32)
                cc1o = dramw.tile([128, 2], F32, addr_space="Shared")
                acc1 = tiny.tile([128, 2], F32, tag="acc")
                nc.vector.tensor_reduce(acc1[:, 0:1], sums1[:], AX.X, AO.add)
                nc.vector.tensor_reduce(acc1[:, 1:2], sums1q[:], AX.X, AO.add)
                nc.sync.dma_start(cc1i[:], acc1[:])
                if NO_CC:
                    nc.sync.dma_start(cc1o[:], cc1i[:])
                else:
                    nc.gpsimd.collective_compute(
                        "AllReduce", AO.add, ins=[cc1i[:]], outs=[cc1o[:]],
                        replica_groups=[list(range(NCORES))])
                g1 = tiny.tile([128, 2], F32, tag="acc")
                nc.sync.dma_start(g1[:], cc1o[:])

                def stats_block(g, gamma, beta, rga, rgam, alpha):
                    # cross-half channel sum on the (idle) PE: out[c] =
                    # g[c%64] + g[c%64+64] via a stacked-identity fp32 matmul
                    totp = ps.tile([128, 2], F32, tag="pstot", bufs=1,
                                   name="pstot")
                    nc.tensor.matmul(totp[:], idents[:], g[:],
                                     start=True, stop=True,
                                     skip_group_check=True)
                    tot = totp
                    mm = tiny.tile([128, 2], F32, tag="acc")
                    nc.vector.tensor_scalar(mm[:], tot[:], 1.0 / NG, None,
                                            AO.mult)
                    mean = mm[:, 0:1]
                    m2 = tiny.tile([128, 1], F32, tag="t1")
                    nc.vector.tensor_tensor(m2[:], mean, mean, AO.mult)
                    var = tiny.tile([128, 1], F32, tag="t1")
                    nc.vector.tensor_tensor(var[:], mm[:, 1:2], m2[:],
                                            AO.subtract)
                    epst = tiny.tile([128, 1], F32, tag="t1")
                    nc.vector.memset(epst[:], EPS)
                    std = tiny.tile([128, 1], F32, tag="t1")
                    nc.scalar.activation(std[:], var[:], AF.Sqrt, bias=epst[:])
                    rstd = tiny.tile([128, 1], F32, tag="t1")
                    nc.vector.reciprocal(rstd[:], std[:])
                    rscv = tiny.tile([128, 1], F32, tag="t1")
                    nc.vector.tensor_tensor(rscv[:], std[:], rgam, AO.mult)
                    sc = tiny.tile([128, 1], F32, tag="t1")
                    nc.vector.tensor_tensor(sc[:], gamma, rstd[:], AO.mult)
                    nmsc = tiny.tile([128, 1], F32, tag="t1")
                    nc.vector.scalar_tensor_tensor(nmsc[:], mean, -1.0, sc[:],
                                                   AO.mult, AO.mult)
                    bi = tiny.tile([128, 1], F32, tag="t1")
                    nc.vector.tensor_tensor(bi[:], beta, nmsc[:], AO.add)
                    stdrg = tiny.tile([128, 1], F32, tag="t1")
                    nc.vector.tensor_tensor(stdrg[:], std[:], rga, AO.mult)
                    nbst = tiny.tile([128, 1], F32, tag="t1")
                    nc.vector.scalar_tensor_tensor(nbst[:], bi[:], -alpha,
                                                   stdrg[:], AO.mult, AO.mult)
                    th = tiny.tile([128, 1], F32, tag="t1")
                    nc.vector.tensor_tensor(th[:], stdrg[:], nbst[:], AO.add)
                    bstd = tiny.tile([128, 1], F32, tag="t1")
                    nc.vector.tensor_tensor(bstd[:], bi[:], std[:], AO.mult)
                    gamv = tiny.tile([128, 1], F32, tag="t1")
                    nc.vector.tensor_tensor(gamv[:], bstd[:], rgam, AO.mult)
                    gmw = tiny.tile([128, 1], F32, tag="t1")
                    nc.vector.tensor_scalar(gmw[:], gamv[:], 1.0 - alpha, None,
                                            AO.mult)
                    return th, gamv, rscv, gmw

                th1, gm1, _rsc1, gmw1 = stats_block(
                    g1, cpars[:, 0:1], cpars[:, 1:2], cpars[:, 4:5],
                    cpars[:, 6:7], alpha1)

                # ============ phase B + C: LIF1 + conv2 ============
                # one-time zeroing of spike-plane borders happens implicitly:
                # spike planes reuse the "pl" tag whose buffers held fully
                # DMA-written x planes with zero borders; interior writes and
                # the half-dup DMA never touch the borders afterwards.
                y2s = [None] * NPAIR
                for bp in range(2):
                    Pprev = [None] * NQ
                    for t in range(1, 5):
                        p = (t - 1) * 2 + bp
                        spl = []
                        for j in range(2):
                            sp = plpool.tile([128, PP], F16, tag="pl")
                            spl.append(sp)
                        spAr = spl[0].rearrange("p (r w) -> p r w", w=HP)
                        spBr = spl[1].rearrange("p (r w) -> p r w", w=HP)
                        qas = []
                        for hq in range(NQ):
                            off = QL * hq
                            ysl = y1s[p][:, off:off + QL]
                            if t == 1:
                                qa = ysl
                            else:
                                q = hf.tile([128, QL], F32, tag="q2", bufs=4)
                                nc.gpsimd.tensor_tensor(q[:], ysl,
                                                        Pprev[hq][:], AO.add)
                                qa = q[:]
                            qas.append(qa)
                            qa3 = qa.rearrange("p (r w) -> p r w", w=W)
                            dstA = spAr[0:64, 1 + 14 * hq:15 + 14 * hq,
                                        1:1 + W]
                            nc.vector.tensor_scalar(dstA, qa3[0:64],
                                                    th1[0:64], None, AO.is_ge)
                        # image A plane complete first so j=0 matmuls start
                        nc.scalar.dma_start(spl[0][64:128, :], spl[0][0:64, :])
                        for hq in range(NQ):
                            qa3 = qas[hq].rearrange("p (r w) -> p r w", w=W)
                            dstB = spBr[64:128, 1 + 14 * hq:15 + 14 * hq,
                                        1:1 + W]
                            nc.vector.tensor_scalar(dstB, qa3[64:128],
                                                    th1[64:128], None,
                                                    AO.is_ge)
                        nc.scalar.dma_start(spl[1][0:64, :], spl[1][64:128, :])
                        if t < 4:
                            for hq in range(NQ):
                                qa = qas[hq]
                                sb = hf.tile([128, QL], F16, tag="sb", bufs=2)
                                nc.vector.tensor_scalar(sb[:], qa, th1[:],
                                                        None, AO.is_lt)
                                wv = hf.tile([128, QL], F32, tag="wv", bufs=2)
                                nc.scalar.activation(wv[:], qa, AF.Identity,
                                                     bias=gmw1[:],
                                                     scale=1.0 - alpha1)
                                Pn = hf.tile([128, QL], F32, tag="pp", bufs=5)
                                nc.gpsimd.tensor_tensor(Pn[:], wv[:], sb[:],
                                                        AO.mult)
                                Pprev[hq] = Pn
                        strip2 = yspool.tile([128, PIX], F32, tag=f"ys{p}",
                                             bufs=1)
                        y2s[p] = strip2
                        conv2_pair(spl[0], spl[1], strip2, p)

                # ---- stats2 allreduce
                cc2i = dramw.tile([128, 2], F32)
                cc2o = dramw.tile([128, 2], F32, addr_space="Shared")
                acc2 = tiny.tile([128, 2], F32, tag="acc")
                nc.vector.tensor_reduce(acc2[:, 0:1], sums2[:], AX.X, AO.add)
                nc.vector.tensor_reduce(acc2[:, 1:2], sums2q[:], AX.X, AO.add)
                nc.sync.dma_start(cc2i[:], acc2[:])
                if NO_CC:
                    nc.sync.dma_start(cc2o[:], cc2i[:])
                else:
                    nc.gpsimd.collective_compute(
                        "AllReduce", AO.add, ins=[cc2i[:]], outs=[cc2o[:]],
                        replica_groups=[list(range(NCORES))])
                g2 = tiny.tile([128, 2], F32, tag="acc")
                nc.sync.dma_start(g2[:], cc2o[:])
                th2, gm2, rsc2, gmw2 = stats_block(
                    g2, cpars[:, 2:3], cpars[:, 3:4], cpars[:, 5:6],
                    cpars[:, 7:8], alpha2)

                # ============ phase D: residual + LIF2 ============
                # flat t-outer iteration list, software-pipelined: xs loads
                # are emitted K iterations ahead on SP, and out-DMAs also go
                # on SP *behind* the prefetched loads, so their wait on `ot`
                # never blocks loads or any compute queue.
                Pprev2 = {0: [None] * NQ, 1: [None] * NQ}
                iters = [(t, bp, hq) for t in range(1, 5)
                         for bp in range(2) for hq in range(NQ)]
                KPF = 6
                xstiles = {}

                def issue_load(idx):
                    t, bp, hq = iters[idx]
                    iA = (t - 1) * 4 + bp * 2
                    off = QL * hq
                    xs = hf.tile([128, QL], F32, tag="xs", bufs=8)
                    nc.sync.dma_start(xs[:],
                                      xin[iA:iA + 2, :, off:off + QL])
                    xstiles[idx] = xs

                for idx in range(KPF):
                    issue_load(idx)
                for idx, (t, bp, hq) in enumerate(iters):
                    if idx + KPF < len(iters):
                        issue_load(idx + KPF)
                    p = (t - 1) * 2 + bp
                    iA = (t - 1) * 4 + bp * 2
                    off = QL * hq
                    xs = xstiles.pop(idx)
                    # q = xs*rsc2 + y2 (+ P), built in place in xs
                    nc.scalar.activation(xs[:], xs[:], AF.Copy,
                                         scale=rsc2[:])
                    if t < 4:
                        nc.vector.tensor_tensor(
                            xs[:], xs[:], y2s[p][:, off:off + QL], AO.add)
                    else:
                        nc.gpsimd.tensor_tensor(
                            xs[:], xs[:], y2s[p][:, off:off + QL], AO.add)
                    if t > 1:
                        eng = nc.vector if (t == 4 or hq % 2) else nc.gpsimd
                        eng.tensor_tensor(xs[:], xs[:],
                                          Pprev2[bp][hq][:], AO.add)
                    q2v = xs[:]
                    ot = hf.tile([128, QL], F16, tag="ot", bufs=2)
                    nc.vector.tensor_scalar(ot[:], q2v, th2[:],
                                            None, AO.is_ge)
                    if t < 4:
                        sb2 = hf.tile([128, QL], F16, tag="sb", bufs=2)
                        nc.vector.tensor_scalar(sb2[:], ot[:], 0.0,
                                                None, AO.is_equal)
                        wv2 = hf.tile([128, QL], F32, tag="wv", bufs=2)
                        nc.scalar.activation(wv2[:], q2v, AF.Identity,
                                             bias=gmw2[:],
                                             scale=1.0 - alpha2)
                        ptag = ("pp", 5) if bp == 0 else ("q2", 4)
                        Pn = hf.tile([128, QL], F32, tag=ptag[0],
                                     bufs=ptag[1])
                        peng = nc.vector if hq % 2 else nc.gpsimd
                        peng.tensor_tensor(Pn[:], wv2[:], sb2[:],
                                           AO.mult)
                        Pprev2[bp][hq] = Pn
                    oeng = nc.scalar if idx % 2 else nc.sync
                    oeng.dma_start(outp[iA:iA + 2, :, off:off + QL],
                                   ot[:])

    nc.compile()
    return nc, names


def _sigmoid(x):
    return 1.0 / (1.0 + np.exp(-float(x)))


def prepare(x, conv1_w, bn1_gamma, bn1_beta, lif1_w, conv2_w, bn2_gamma,
            bn2_beta, lif2_w):
    x = np.ascontiguousarray(np.asarray(x, np.float32))
    conv1_w = np.asarray(conv1_w, np.float32)
    conv2_w = np.asarray(conv2_w, np.float32)

    a1 = _sigmoid(np.asarray(lif1_w).reshape(-1)[0])
    a2 = _sigmoid(np.asarray(lif2_w).reshape(-1)[0])

    key = (round(a1, 12), round(a2, 12))
    if key not in _prog_cache:
        _prog_cache[key] = _build(a1, a2)
    nc, names = _prog_cache[key]

    # fp16 hi/lo split of x, padded planes (encoding only; exact split)
    xh = x.astype(np.float16)
    xl = (x - xh.astype(np.float32)).astype(np.float16)
    xpad = np.zeros((T, B, C, 2, HP, HP), np.float16)
    xpad[:, :, :, 0, 1:57, 1:57] = xh
    xpad[:, :, :, 1, 1:57, 1:57] = xl
    xpad = np.ascontiguousarray(xpad.transpose(0, 1, 3, 2, 4, 5))  # t,b,2,c,hp,hp

    # [xh ; xh shifted left one column (flat)] planes for the paired lo pass
    xhf = xpad[:, :, 0].reshape(T, B, C, PP)
    xbs = np.zeros((T, B, 2, C, PP), np.float16)
    xbs[:, :, 0] = xhf
    xbs[:, :, 1, :, :PP - 1] = xhf[:, :, :, 1:]

    w1h = conv1_w.astype(np.float16)
    w1l = (conv1_w - w1h.astype(np.float32)).astype(np.float16)
    w2h = conv2_w.astype(np.float16)
    w2l = (conv2_w - w2h.astype(np.float32)).astype(np.float16)

    def tapstack(wtop, wbot):
        out = np.zeros((128, 9 * 64), np.float16)
        for a in range(9):
            di, dj = a // 3, a % 3
            out[0:64, a * 64:(a + 1) * 64] = wtop[:, :, di, dj].T
            out[64:128, a * 64:(a + 1) * 64] = wbot[:, :, di, dj].T
        return out

    w1a_np = tapstack(w1h, w1h)
    w2a_np = tapstack(w2h, w2l)
    w1p_np = np.zeros((128, 3 * 64), np.float16)
    w1s_np = np.zeros((128, 3 * 64), np.float16)
    for di in range(3):
        w1p_np[0:64, di * 64:(di + 1) * 64] = w1l[:, :, di, 0].T
        w1p_np[64:128, di * 64:(di + 1) * 64] = w1l[:, :, di, 1].T
        w1s_np[0:64, di * 64:(di + 1) * 64] = w1l[:, :, di, 2].T

    def dup(v):
        v = np.asarray(v, np.float32).reshape(64)
        return np.concatenate([v, v])

    cpar_np = np.zeros((128, 8), np.float32)
    cpar_np[:, 0] = dup(bn1_gamma)
    cpar_np[:, 1] = dup(bn1_beta)
    cpar_np[:, 2] = dup(bn2_gamma)
    cpar_np[:, 3] = dup(bn2_beta)
    cpar_np[:, 4] = 1.0 / (a1 * dup(bn1_gamma))
    cpar_np[:, 5] = 1.0 / (a2 * dup(bn2_gamma))
    cpar_np[:, 6] = 1.0 / dup(bn1_gamma)
    cpar_np[:, 7] = 1.0 / dup(bn2_gamma)

    kk, mm_ = np.meshgrid(np.arange(128), np.arange(128), indexing='ij')
    ident_np = (kk % 64 == mm_ % 64).astype(np.float32)

    in_maps = []
    for k in range(NCORES):
        xta_np = np.ascontiguousarray(
            xpad[:, 4 * k:4 * k + 4].reshape(NIMG, 2, 64, PP))
        xtb_np = np.ascontiguousarray(
            xbs[:, 4 * k:4 * k + 4].reshape(NIMG, 2, 64, PP))
        xin_np = np.ascontiguousarray(
            x[:, 4 * k:4 * k + 4].reshape(NIMG, 64, PIX))
        in_maps.append({
            names['xta']: xta_np,
            names['xtb']: xtb_np,
            names['xin']: xin_np,
            names['w1a']: w1a_np,
            names['w1p']: w1p_np,
            names['w1s']: w1s_np,
            names['w2a']: w2a_np,
            names['cpar']: cpar_np,
            names['ident']: ident_np,
        })

    return nc, names, in_maps


def kernel(**inputs):
    from concourse.bass_utils import run_bass_kernel_spmd
    nc, names, in_maps = prepare(**inputs)
    res = run_bass_kernel_spmd(nc, in_maps, core_ids=list(range(NCORES)))
    global LAST_RES, LAST_NAMES
    LAST_RES, LAST_NAMES = res, names
    out = np.empty((T, B, C, H, W), np.float32)
    for k in range(NCORES):
        o = res.results[k][names['outp']]
        out[:, 4 * k:4 * k + 4] = o.reshape(T, BL, C, H, W).astype(np.float32)
    return out


if __name__ == "__main__":
    rng = np.random.default_rng(0)
    xs = rng.standard_normal((T, B, C, H, W)).astype(np.float32)
    w1 = (rng.standard_normal((64, 64, 3, 3)) * 0.05).astype(np.float32)
    w2 = (rng.standard_normal((64, 64, 3, 3)) * 0.05).astype(np.float32)
    o = kernel(x=xs, conv1_w=w1, bn1_gamma=np.ones(64, np.float32),
               bn1_beta=np.zeros(64, np.float32),
               lif1_w=np.zeros(1, np.float32), conv2_w=w2,
               bn2_gamma=np.ones(64, np.float32),
               bn2_beta=np.zeros(64, np.float32),
               lif2_w=np.zeros(1, np.float32))
    print("ran:", o.shape, float(o.mean()))
